# revision 9
# baseline (speedup 1.0000x reference)
"""3-layer GAT (PPI-style) forward on 8 Trainium2 NeuronCores — v4.

Strategy (SPMD, one NEFF on 8 cores):
  - Host: degree-balanced node permutation into 8 cores x 2500 nodes
    (tiles of 128 dst nodes), per-core tiles sorted by edge load; edges
    (WITHOUT self-loops) sorted by dst, chunked to 128-edge chunks with
    per-tile ragged chunk counts (padded even for fp8 DoubleRow pairs);
    int16 gather indices, fp8 one-hot scatter matrices in DoubleRow pair
    layout, bf16 transposed one-hots for the ed matmuls.
  - Payload AllGather split in two (tiles [0,S) / [S,T)) on separate
    Shared tensors; edges src-partitioned into the groups.
  - Dense phase per tile: [h | lin] = x @ [W | Wl] (bf16 PE); es/ed dots
    folded into the matmul; payload row [h (fp8) | es (f32)]; running
    per-core es-max tracked; after the dense loop a tiny AllGather + per-
    tile ed-max gives B_t,h = max(es_max + ed_max, 0) - DELTA, an upper
    bound on the attention logits used to shift exp() into fp8 range
    (shift cancels exactly in the softmax).
  - Aggregation per tile: ed via ohT^T @ edt; dma_gather per group;
    w8 = fp8(exp(leaky(es+ed) - B)); numerator AND denominator use the
    same w8 (quantization cancels in alpha): Gw8 = fp8(payload_h * w8),
    scatter-sum via fp8 DoubleRow matmuls (256-edge pairs, 0.5 cyc/col)
    accumulating [128,HC] + denominator [128,H] in PSUM; self-loop terms
    added in f32 in the epilogue (exact anchor for the denominator);
    normalize, skip+bias, ELU; PE-transpose feeds the next dense phase.
"""

import math
import numpy as np

N_CORES = 8
GROUP = 6          # max gather chunks per dma_gather call (even)
SPLIT_TILE = 16    # tiles [0, S) -> AG group A, [S, T) -> group B
DELTA = 4.0        # exp() headroom above the logit bound (fp8 range ctrl)


# --------------------------------------------------------------------------
# host-side prep
# --------------------------------------------------------------------------

def _balance_permutation(dst, n, n_cores, tiles_per_core, rows_last):
    import heapq

    deg = np.bincount(dst, minlength=n).astype(np.int64)
    order = np.argsort(-deg, kind="stable")
    n_tiles = n_cores * tiles_per_core
    caps = np.full(n_tiles, 128, np.int64)
    caps[tiles_per_core - 1 :: tiles_per_core] = rows_last
    heap = [(0, int(b)) for b in range(n_tiles)]
    heapq.heapify(heap)
    members = [[] for _ in range(n_tiles)]
    loads = np.zeros(n_tiles, np.int64)
    for node in order:
        while True:
            load, b = heapq.heappop(heap)
            if len(members[b]) < caps[b]:
                break
        members[b].append(node)
        loads[b] += deg[node]
        if len(members[b]) < caps[b]:
            heapq.heappush(heap, (int(loads[b]), b))

    perm_o2n = np.empty(n, np.int64)
    per_core = tiles_per_core * 128 - (128 - rows_last)
    for c in range(n_cores):
        bs = list(range(c * tiles_per_core, (c + 1) * tiles_per_core))
        full, short = bs[:-1], bs[-1]
        full.sort(key=lambda b: -loads[b])
        for t, b in enumerate(full + [short]):
            base = c * per_core + t * 128
            ids = np.asarray(members[b], np.int64)
            perm_o2n[ids] = base + np.arange(len(ids))
    return perm_o2n


def _wrap16_rep(a):
    w = a.reshape(-1, 16).T.astype(np.int16)
    return np.ascontiguousarray(np.tile(w, (8, 1)))


def _host_prep(inputs, n_cores=N_CORES):
    import ml_dtypes

    bf16 = ml_dtypes.bfloat16
    x = np.asarray(inputs["x"], np.float32)
    ei = np.asarray(inputs["edge_index"])
    n, f_in = x.shape
    src = ei[0].astype(np.int64)      # self-loops handled in the epilogue
    dst = ei[1].astype(np.int64)

    per_core = n // n_cores
    T = math.ceil(per_core / 128)
    rows_last = per_core - (T - 1) * 128
    S = SPLIT_TILE
    rowsA = S * 128
    rowsB = per_core - rowsA

    perm = _balance_permutation(dst, n, n_cores, T, rows_last)

    local = perm % per_core
    core = perm // per_core
    in_a = local < rowsA
    grow = np.where(in_a, core * rowsA + local, core * rowsB + (local - rowsA))

    dst_n = perm[dst]
    src_g = (~in_a[src]).astype(np.int64)      # 0 = group A, 1 = group B
    src_row = grow[src]

    core_of = dst_n // per_core
    loc_of = dst_n % per_core

    per_ctg = {}
    counts = np.zeros((n_cores, T, 2), np.int64)
    for c in range(n_cores):
        sel = core_of == c
        s, loc, g = src_row[sel], loc_of[sel], src_g[sel]
        o = np.argsort(loc, kind="stable")
        s, loc, g = s[o], loc[o], g[o]
        tile_of = loc // 128
        for t in range(T):
            mt = tile_of == t
            for gg in range(2):
                m = mt & (g == gg)
                per_ctg[c, t, gg] = (s[m], loc[m] - t * 128)
                counts[c, t, gg] = m.sum()

    # per-tile ragged chunk counts (even, for DoubleRow pairs)
    def even_ceil(v):
        k = max(2, math.ceil(v / 128))
        return k + (k & 1)

    nchunks = [(even_ceil(counts[:, t, 0].max()), even_ceil(counts[:, t, 1].max()))
               for t in range(T)]
    ntot = [a + b for a, b in nchunks]

    co_idx = np.concatenate([[0], np.cumsum([v * 8 for v in ntot])])
    co_oh = np.concatenate([[0], np.cumsum([v * 128 for v in ntot])])

    src16 = np.zeros((n_cores, 128, int(co_idx[-1])), np.int16)
    oh8 = np.zeros((n_cores, 128, int(co_oh[-1])), np.uint8)   # fp8e4m3 bits
    ohTs = np.zeros((n_cores, 128, int(co_oh[-1])), bf16)
    ONE8 = 0x38                                                # 1.0 in e4m3
    pp = np.arange(128)
    for c in range(n_cores):
        for t in range(T):
            nA, nB = nchunks[t]
            cap = (nA + nB) * 128
            ps = np.zeros(cap, np.int64)
            pl = np.full(cap, -1, np.int64)
            for gg, off, ncg in ((0, 0, nA), (1, nA * 128, nB)):
                s_, l_ = per_ctg[c, t, gg]
                e = len(s_)
                ps[off : off + e] = s_
                pl[off : off + e] = l_
            src16[c, :, co_idx[t] : co_idx[t + 1]] = _wrap16_rep(ps)
            dl = pl.reshape(nA + nB, 128)
            for cch in range(nA + nB):
                m = dl[cch] >= 0
                j = dl[cch, m]
                # DoubleRow pair layout: pair p = cch//2, sub i = cch%2,
                # cols [p*256 + i*128 + j]
                base = co_oh[t] + (cch // 2) * 256 + (cch % 2) * 128
                oh8[c, pp[m], base + j] = ONE8
                ohTs[c, j, co_oh[t] + cch * 128 + pp[m]] = 1

    rows_pad = T * 128
    x_perm = np.zeros((n, f_in), np.float32)
    x_perm[perm] = x
    xT = []
    for c in range(n_cores):
        blk = np.zeros((rows_pad, f_in), np.float32)
        blk[:per_core] = x_perm[c * per_core : (c + 1) * per_core]
        xT.append(np.ascontiguousarray(blk.T).astype(bf16))

    g = lambda k: np.asarray(inputs[k], np.float32)
    h1, c1 = g("a1s").shape
    h3, c3 = g("a3s").shape
    d1 = h1 * c1

    def fold(Wk, ak_s, ak_d, h, cc):
        W = g(Wk)
        a_s, a_d = g(ak_s), g(ak_d)
        waS = np.stack([W[:, i * cc : (i + 1) * cc] @ a_s[i] for i in range(h)], 1)
        waD = np.stack([W[:, i * cc : (i + 1) * cc] @ a_d[i] for i in range(h)], 1)
        return np.concatenate([waS, waD], 1).astype(bf16)  # [din, 2h]

    waug1 = np.concatenate([g("W1"), g("Wl1")], 1).astype(bf16)
    waug2 = np.concatenate([g("W2"), g("Wl2")], 1).astype(bf16)
    waug3 = np.concatenate([g("W3"), g("Wl3")], 1).astype(bf16)
    wsd1 = fold("W1", "a1s", "a1d", h1, c1)
    wsd2 = fold("W2", "a2s", "a2d", h1, c1)
    wsd3 = fold("W3", "a3s", "a3d", h3, c3)

    rep = lambda v: np.ascontiguousarray(
        np.broadcast_to(v[None, :], (128, v.shape[0]))
    ).astype(np.float32)
    base = dict(
        waug1=waug1, waug2=waug2, waug3=waug3,
        wsd1=wsd1, wsd2=wsd2, wsd3=wsd3,
        bsum1=rep(g("b1") + g("bl1")),
        bsum2=rep(g("b2") + g("bl2")),
        bsum3=rep(g("b3") + g("bl3")),
        idmat=np.eye(128, dtype=bf16),
    )
    in_maps = []
    for c in range(n_cores):
        m = dict(base)
        m["xT1"] = xT[c]
        m["src16"] = src16[c]
        m["oh8"] = np.ascontiguousarray(oh8[c])
        m["ohT"] = np.ascontiguousarray(ohTs[c])
        in_maps.append(m)

    cfg = dict(
        n=n, f_in=f_in, n_cores=n_cores, per_core=per_core,
        tiles_per_core=T, rows_last=rows_last, rows_pad=rows_pad,
        s_tile=S, rows_a=rowsA, rows_b=rowsB,
        nchunks=nchunks, co_idx=[int(v) for v in co_idx],
        co_oh=[int(v) for v in co_oh],
        h1=h1, c1=c1, d1=d1, h3=h3, c3=c3,
    )
    return in_maps, cfg, perm


# --------------------------------------------------------------------------
# bass program
# --------------------------------------------------------------------------

def _layer_dims(cfg):
    """Payload row: [h fp8 (HC bytes, 4-pad)] [es f32 (4H bytes)], row size
    a 256B multiple (dma_gather restriction)."""
    out = []
    for li in (1, 2, 3):
        if li < 3:
            h, c = cfg["h1"], cfg["c1"]
            din = cfg["f_in"] if li == 1 else cfg["d1"]
            nlin = cfg["d1"]
        else:
            h, c = cfg["h3"], cfg["c3"]
            din = cfg["d1"]
            nlin = cfg["c3"]
        hc = h * c
        esb = math.ceil(hc / 4) * 4        # byte offset of es
        pw = math.ceil((esb + 4 * h) / 256) * 256
        kch = math.ceil(din / 128)
        out.append(dict(li=li, din=din, kch=kch, hc=hc, nlin=nlin,
                        h=h, c=c, es4=esb // 4, pw=pw, naug=hc + nlin))
    return out


def _groups(n, grp):
    out = []
    o = 0
    while o < n:
        out.append((o, min(grp, n - o)))
        o += grp
    return out


def _build(cfg):
    import concourse.bass as bass
    import concourse.bacc as bacc
    import concourse.mybir as mybir
    import concourse.tile as tile
    from contextlib import ExitStack

    f32 = mybir.dt.float32
    bf = mybir.dt.bfloat16
    i16 = mybir.dt.int16
    u8 = mybir.dt.uint8
    f8 = mybir.dt.float8e4
    EXP = mybir.ActivationFunctionType.Exp
    CPY = mybir.ActivationFunctionType.Copy
    ALU = mybir.AluOpType
    DR = mybir.MatmulPerfMode.DoubleRow

    n_cores = cfg["n_cores"]
    n = cfg["n"]
    T = cfg["tiles_per_core"]
    rows_last = cfg["rows_last"]
    per_core = cfg["per_core"]
    S = cfg["s_tile"]
    rowsA, rowsB = cfg["rows_a"], cfg["rows_b"]
    NCH = cfg["nchunks"]
    CO_IDX = cfg["co_idx"]
    CO_OH = cfg["co_oh"]
    NMAX = max(a + b for a, b in NCH)
    GRP = GROUP
    layers = _layer_dims(cfg)

    nc = bacc.Bacc(None, target_bir_lowering=False, num_swdge_queues=2)

    # ---- parameters -----------------------------------------------------
    xT1 = nc.declare_dram_parameter("xT1", [cfg["f_in"], T * 128], bf, isOutput=False)
    waug_p, wsd_p, bsum_p = {}, {}, {}
    for L in layers:
        li = L["li"]
        waug_p[li] = nc.declare_dram_parameter(
            f"waug{li}", [L["din"], L["naug"]], bf, isOutput=False)
        wsd_p[li] = nc.declare_dram_parameter(
            f"wsd{li}", [L["din"], 2 * L["h"]], bf, isOutput=False)
        bsum_p[li] = nc.declare_dram_parameter(
            f"bsum{li}", [128, L["nlin"]], f32, isOutput=False)
    src16_p = nc.declare_dram_parameter("src16", [128, CO_IDX[-1]], i16, isOutput=False)
    oh_p = nc.declare_dram_parameter("oh8", [128, CO_OH[-1]], u8, isOutput=False)
    ohT_p = nc.declare_dram_parameter("ohT", [128, CO_OH[-1]], bf, isOutput=False)
    id_p = nc.declare_dram_parameter("idmat", [128, 128], bf, isOutput=False)
    out_p = nc.declare_dram_parameter("out", [per_core, cfg["c3"]], f32, isOutput=True)

    with tile.TileContext(nc, num_cores=n_cores) as tc, ExitStack() as ctx:
        # ---- dram scratch ----------------------------------------------
        dram = ctx.enter_context(tc.tile_pool(name="dram", bufs=1, space="DRAM"))
        pshard = {L["li"]: dram.tile([per_core, L["pw"]], u8, tag=f"pshard{L['li']}",
                                     name=f"pshard{L['li']}") for L in layers}
        pfullA = {L["li"]: dram.tile([n_cores * rowsA, L["pw"]], u8,
                                     tag=f"pfa{L['li']}", name=f"pfa{L['li']}",
                                     addr_space="Shared") for L in layers}
        pfullB = {L["li"]: dram.tile([n_cores * rowsB, L["pw"]], u8,
                                     tag=f"pfb{L['li']}", name=f"pfb{L['li']}",
                                     addr_space="Shared") for L in layers}
        linb = {L["li"]: dram.tile([T * 128, L["nlin"]], f32, tag=f"lin{L['li']}",
                                   name=f"lin{L['li']}") for L in layers}
        esd = {L["li"]: dram.tile([1, L["h"]], f32, tag=f"esd{L['li']}",
                                  name=f"esd{L['li']}") for L in layers}
        esg = {L["li"]: dram.tile([n_cores, L["h"]], f32, tag=f"esg{L['li']}",
                                  name=f"esg{L['li']}", addr_space="Shared")
               for L in layers}

        # ---- pools ------------------------------------------------------
        consts = ctx.enter_context(tc.tile_pool(name="consts", bufs=1))
        wtp = ctx.enter_context(tc.tile_pool(name="wtp", bufs=1))
        xTp = ctx.enter_context(tc.tile_pool(name="xTp", bufs=2))
        ptp = ctx.enter_context(tc.tile_pool(name="ptp", bufs=2))
        ltp = ctx.enter_context(tc.tile_pool(name="ltp", bufs=2))
        gp = ctx.enter_context(tc.tile_pool(name="gp", bufs=3))
        gwp = ctx.enter_context(tc.tile_pool(name="gwp", bufs=2))
        idxp = ctx.enter_context(tc.tile_pool(name="idxp", bufs=2))
        ohp = ctx.enter_context(tc.tile_pool(name="ohp", bufs=2))
        lgp = ctx.enter_context(tc.tile_pool(name="lgp", bufs=4))
        epip = ctx.enter_context(tc.tile_pool(name="epip", bufs=1))
        recp = ctx.enter_context(tc.tile_pool(name="recp", bufs=4))
        psum_d = ctx.enter_context(tc.tile_pool(name="psum_d", bufs=1, space="PSUM"))
        psum_a = ctx.enter_context(tc.tile_pool(name="psum_a", bufs=1, space="PSUM"))
        psum_e = ctx.enter_context(tc.tile_pool(name="psum_e", bufs=2, space="PSUM"))
        psum_n = ctx.enter_context(tc.tile_pool(name="psum_n", bufs=1, space="PSUM"))
        psum_t = ctx.enter_context(tc.tile_pool(name="psum_t", bufs=1, space="PSUM"))

        # ---- constants ---------------------------------------------------
        idm = consts.tile([128, 128], bf, tag="idm")
        nc.sync.dma_start(out=idm[:, :], in_=id_p[:, :])
        xT1_sb = consts.tile([cfg["f_in"], T * 128], bf, tag="xT1")
        nc.sync.dma_start(out=xT1_sb[:, :], in_=xT1[:, :])
        wt, wsd, bsum, edts, esmx, bneg = {}, {}, {}, {}, {}, {}
        for L in layers:
            li, KCH, DIN, H = L["li"], L["kch"], L["din"], L["h"]
            for k in range(KCH):
                kk = min(128, DIN - k * 128)
                w = wtp.tile([128, L["naug"]], bf, tag=f"w{li}_{k}", name=f"w{li}_{k}")
                nc.sync.dma_start(out=w[:kk, :], in_=waug_p[li][k * 128 : k * 128 + kk, :])
                wt[li, k] = w
                s = wtp.tile([128, 2 * H], bf, tag=f"s{li}_{k}", name=f"s{li}_{k}")
                nc.sync.dma_start(out=s[:kk, :], in_=wsd_p[li][k * 128 : k * 128 + kk, :])
                wsd[li, k] = s
            b = consts.tile([128, L["nlin"]], f32, tag=f"b{li}", name=f"b{li}")
            nc.sync.dma_start(out=b[:, :], in_=bsum_p[li][:, :])
            bsum[li] = b
            e = consts.tile([128, T * H], bf, tag=f"e{li}", name=f"e{li}")
            edts[li] = e
            em = consts.tile([128, H], f32, tag=f"esm{li}", name=f"esm{li}")
            esmx[li] = em
            bn = consts.tile([128, T * H], f32, tag=f"bn{li}", name=f"bn{li}")
            bneg[li] = bn

        qn = [0]

        def rows_of(t):
            return 128 if t < T - 1 else rows_last

        def chunks(w):
            return [(c0, min(c0 + 512, w)) for c0 in range(0, w, 512)]

        # ---------------- dense phase for one tile -----------------------
        def dense_tile(L, t, get_lhsT):
            li, DIN, KCH = L["li"], L["din"], L["kch"]
            H, C, HC = L["h"], L["c"], L["hc"]
            NLIN, ES4, PW = L["nlin"], L["es4"], L["pw"]
            r = rows_of(t)

            lhsTs = {}

            def lhsT_of(k, kk):
                if k not in lhsTs:
                    lhsTs[k] = get_lhsT(k, kk)
                return lhsTs[k]

            wA = HC + NLIN if li == 3 else HC
            pse = psum_e.tile([128, 2 * H], f32, tag="pe", name="pse")
            pdA = psum_d.tile([128, 1024], f32, tag="pd", name="pdA")
            for k in range(KCH):
                kk = min(128, DIN - k * 128)
                lhsT = lhsT_of(k, kk)
                st, sp = (k == 0), (k == KCH - 1)
                for c0, c1 in chunks(wA):
                    nc.tensor.matmul(pdA[:, c0:c1], lhsT[:kk, :],
                                     wt[li, k][:kk, c0:c1], start=st, stop=sp)
                nc.tensor.matmul(pse[:, :], lhsT[:kk, :], wsd[li, k][:kk, :],
                                 start=st, stop=sp)

            # payload assembly (h in fp8, es in f32)
            pt = ptp.tile([128, PW], u8, tag="pt")
            ptb = pt.bitcast(f8)
            nc.scalar.activation(ptb[:, :HC], pdA[:, :HC], CPY)
            ptf = pt.bitcast(f32)
            nc.scalar.activation(ptf[:, ES4 : ES4 + H], pse[:, :H], CPY)
            nc.scalar.activation(edts[li][:, t * H : (t + 1) * H], pse[:, H : 2 * H], CPY)
            # track per-core es max
            if t == 0:
                nc.vector.tensor_copy(esmx[li][:, :], pse[:, :H])
            else:
                nc.vector.tensor_tensor(out=esmx[li][:, :], in0=esmx[li][:, :],
                                        in1=pse[:, :H], op=ALU.max)
            nc.sync.dma_start(out=pshard[li][t * 128 : t * 128 + r, :], in_=pt[:r, :])
            if li < 3:
                pdB = psum_d.tile([128, 1024], f32, tag="pd", name="pdB")
                for k in range(KCH):
                    kk = min(128, DIN - k * 128)
                    lhsT = lhsT_of(k, kk)
                    st, sp = (k == 0), (k == KCH - 1)
                    for c0, c1 in chunks(NLIN):
                        nc.tensor.matmul(pdB[:, c0:c1], lhsT[:kk, :],
                                         wt[li, k][:kk, HC + c0 : HC + c1],
                                         start=st, stop=sp)
            else:
                pdB = pdA
            loff = HC if li == 3 else 0
            lt = ltp.tile([128, NLIN], f32, tag="lt")
            nc.vector.tensor_tensor(out=lt[:, :], in0=pdB[:, loff : loff + NLIN],
                                    in1=bsum[li][:, :], op=ALU.add)
            nc.sync.dma_start(out=linb[li][t * 128 : t * 128 + r, :], in_=lt[:r, :])

        # ------------- per-layer logit bound B (after dense loop) ---------
        def emit_bound(L):
            li, H = L["li"], L["h"]
            esr = consts.tile([128, H], f32, tag=f"esr{li}", name=f"esr{li}")
            nc.gpsimd.partition_all_reduce(esr[:, :], esmx[li][:, :], 128,
                                           reduce_op=bass.bass_isa.ReduceOp.max)
            nc.sync.dma_start(out=esd[li][:, :], in_=esr[0:1, :])
            nc.gpsimd.collective_compute(
                "AllGather", ALU.bypass,
                replica_groups=[list(range(n_cores))],
                ins=[esd[li].opt()], outs=[esg[li].opt()],
            )
            ess = consts.tile([n_cores, H], f32, tag=f"ess{li}", name=f"ess{li}")
            nc.sync.dma_start(out=ess[:, :], in_=esg[li][:, :])
            essr = consts.tile([n_cores, H], f32, tag=f"essr{li}", name=f"essr{li}")
            nc.gpsimd.partition_all_reduce(essr[:, :], ess[:, :], n_cores,
                                           reduce_op=bass.bass_isa.ReduceOp.max)
            esb_t = consts.tile([128, H], f32, tag=f"esb{li}", name=f"esb{li}")
            nc.gpsimd.partition_broadcast(esb_t[:, :], essr[0:1, :])
            edm = consts.tile([128, T * H], bf, tag=f"edm{li}", name=f"edm{li}")
            nc.gpsimd.partition_all_reduce(edm[:, :], edts[li][:, :], 128,
                                           reduce_op=bass.bass_isa.ReduceOp.max)
            bn = bneg[li]
            nc.vector.tensor_tensor(
                out=bn.rearrange("p (t h) -> p t h", h=H),
                in0=edm.rearrange("p (t h) -> p t h", h=H),
                in1=esb_t.unsqueeze(1).broadcast_to([128, T, H]), op=ALU.add)
            nc.vector.tensor_scalar(out=bn[:, :], in0=bn[:, :], scalar1=0.0,
                                    scalar2=None, op0=ALU.max)
            nc.vector.tensor_scalar(out=bn[:, :], in0=bn[:, :], scalar1=-1.0,
                                    scalar2=DELTA, op0=ALU.mult, op1=ALU.add)

        # ---------------- aggregation for one tile ------------------------
        def agg_tile(L, t):
            li = L["li"]
            H, C, HC = L["h"], L["c"], L["hc"]
            NLIN, ES4, PW = L["nlin"], L["es4"], L["pw"]
            r = rows_of(t)
            nA, nB = NCH[t]
            NT = nA + nB
            NPAIR = NT // 2

            s16 = idxp.tile([128, NMAX * 8], i16, tag="s16")
            nc.sync.dma_start(out=s16[:, : NT * 8],
                              in_=src16_p[:, CO_IDX[t] : CO_IDX[t + 1]])
            oh_sb = ohp.tile([128, NMAX * 128], u8, tag="oh")
            nc.sync.dma_start(out=oh_sb[:, : NT * 128],
                              in_=oh_p[:, CO_OH[t] : CO_OH[t + 1]])
            oh8 = oh_sb.bitcast(f8)
            ohT_sb = ohp.tile([128, NMAX * 128], bf, tag="ohT")
            nc.sync.dma_start(out=ohT_sb[:, : NT * 128],
                              in_=ohT_p[:, CO_OH[t] : CO_OH[t + 1]])

            pe = psum_e.tile([128, NMAX * H], f32, tag="pe", name="pe")
            for cch in range(NT):
                nc.tensor.matmul(pe[:, cch * H : (cch + 1) * H],
                                 ohT_sb[:, cch * 128 : (cch + 1) * 128],
                                 edts[li][:, t * H : (t + 1) * H],
                                 start=True, stop=True)

            pa = psum_a.tile([128, HC], f32, tag="pa", name="pa")
            den = psum_n.tile([128, H], f32, tag="den", name="den")
            bnB = bneg[li][:, t * H : (t + 1) * H]
            glist = [(pfullA[li], o, sz) for o, sz in _groups(nA, GRP)]
            glist += [(pfullB[li], nA + o, sz) for o, sz in _groups(nB, GRP)]
            for src_t, goff, gsz in glist:
                G = gp.tile([128, GRP, PW], u8, tag="G")
                nc.gpsimd.dma_gather(
                    out_ap=G[:, :gsz, :],
                    in_ap=src_t[:, :],
                    idxs_ap=s16[:, goff * 8 : (goff + gsz) * 8],
                    num_idxs=gsz * 128,
                    num_idxs_reg=gsz * 128,
                    elem_size=PW,
                    queue_num=qn[0] % 2,
                )
                qn[0] += 1
                Gf = G.bitcast(f32)
                tl = lgp.tile([128, GRP, H], f32, tag="tl")
                wf = lgp.tile([128, GRP, H], f32, tag="wf")
                nc.vector.tensor_tensor(
                    out=tl[:, :gsz, :], in0=Gf[:, :gsz, ES4 : ES4 + H],
                    in1=pe[:, goff * H : (goff + gsz) * H].rearrange(
                        "p (g h) -> p g h", h=H),
                    op=ALU.add,
                )
                nc.vector.scalar_tensor_tensor(
                    out=wf[:, :gsz, :], in0=tl[:, :gsz, :], scalar=0.2,
                    in1=tl[:, :gsz, :], op0=ALU.mult, op1=ALU.max,
                )
                nc.vector.tensor_tensor(
                    out=wf[:, :gsz, :], in0=wf[:, :gsz, :],
                    in1=bnB.unsqueeze(1).broadcast_to([128, gsz, H]), op=ALU.add)
                web8 = lgp.tile([128, GRP, H], f8, tag="web8")
                nc.scalar.activation(web8[:, :gsz, :], wf[:, :gsz, :], EXP)
                G8 = G.bitcast(f8)[:, :gsz, :HC].rearrange(
                    "p g (h c) -> p g h c", h=H)
                webB = web8[:, :gsz].unsqueeze(3).broadcast_to([128, gsz, H, C])
                Gw8 = gwp.tile([128, GRP, HC], f8, tag="Gw8")
                nc.vector.tensor_tensor(
                    out=Gw8[:, :gsz].rearrange("p g (h c) -> p g h c", h=H),
                    in0=G8, in1=webB, op=ALU.mult)
                for c2 in range(gsz // 2):
                    p = goff // 2 + c2
                    lhsT = oh8[:, p * 256 : (p + 1) * 256].rearrange(
                        "q (two m) -> q two m", two=2)
                    st, sp = (p == 0), (p == NPAIR - 1)
                    for c0, c1 in chunks(HC):
                        nc.tensor.matmul(
                            pa[:, c0:c1], lhsT,
                            Gw8[:, 2 * c2 : 2 * c2 + 2, c0:c1],
                            start=st, stop=sp, perf_mode=DR)
                    nc.tensor.matmul(den[:, :], lhsT,
                                     web8[:, 2 * c2 : 2 * c2 + 2, :],
                                     start=st, stop=sp, perf_mode=DR)

            # ---- epilogue ----
            pac = epip.tile([128, HC], f32, tag="pac", bufs=2)
            nc.scalar.activation(pac[:, :], pa[:, :], CPY)
            dnc = recp.tile([128, H], f32, tag="dnc")
            nc.scalar.activation(dnc[:, :], den[:, :], CPY)
            # self-loop terms in f32 (exact anchor)
            pself = ptp.tile([128, PW], u8, tag="pself")
            nc.sync.dma_start(out=pself[:r, :], in_=pshard[li][t * 128 : t * 128 + r, :])
            ps8 = pself.bitcast(f8)
            psf = pself.bitcast(f32)
            tls = recp.tile([128, H], f32, tag="tls")
            nc.vector.tensor_tensor(out=tls[:r, :], in0=psf[:r, ES4 : ES4 + H],
                                    in1=edts[li][:r, t * H : (t + 1) * H], op=ALU.add)
            nc.vector.scalar_tensor_tensor(out=tls[:r, :], in0=tls[:r, :], scalar=0.2,
                                           in1=tls[:r, :], op0=ALU.mult, op1=ALU.max)
            nc.vector.tensor_tensor(out=tls[:r, :], in0=tls[:r, :], in1=bnB[:r, :],
                                    op=ALU.add)
            ws = recp.tile([128, H], f32, tag="ws")
            nc.scalar.activation(ws[:r, :], tls[:r, :], EXP)
            dent = recp.tile([128, H], f32, tag="dent")
            nc.vector.tensor_tensor(out=dent[:r, :], in0=dnc[:r, :], in1=ws[:r, :],
                                    op=ALU.add)
            nc.vector.tensor_scalar(out=dent[:, :], in0=dent[:, :], scalar1=1e-30,
                                    scalar2=None, op0=ALU.max)
            rec = recp.tile([128, H], f32, tag="rec")
            nc.vector.reciprocal(rec[:, :], dent[:, :])
            xt = epip.tile([128, HC], f32, tag="xt", bufs=2)
            for h in range(H):
                nc.vector.scalar_tensor_tensor(
                    out=xt[:r, h * C : (h + 1) * C], in0=ps8[:r, h * C : (h + 1) * C],
                    scalar=ws[:r, h : h + 1], in1=pac[:r, h * C : (h + 1) * C],
                    op0=ALU.mult, op1=ALU.add)
                nc.vector.tensor_scalar(
                    out=xt[:, h * C : (h + 1) * C], in0=xt[:, h * C : (h + 1) * C],
                    scalar1=rec[:, h : h + 1], scalar2=None, op0=ALU.mult,
                )
            lt2 = ltp.tile([128, NLIN], f32, tag="lt2")
            nc.sync.dma_start(out=lt2[:r, :], in_=linb[li][t * 128 : t * 128 + r, :])
            if li < 3:
                u = epip.tile([128, HC], f32, tag="u")
                e = epip.tile([128, HC], f32, tag="e")
                xo = epip.tile([128, HC], bf, tag="xo")
                if r < 128:
                    nc.vector.memset(xo[:, :], 0)
                nc.vector.tensor_tensor(out=xt[:r, :], in0=xt[:r, :], in1=lt2[:r, :],
                                        op=ALU.add)
                nc.vector.tensor_scalar(out=u[:r, :], in0=xt[:r, :], scalar1=0.0,
                                        scalar2=None, op0=ALU.min)
                nc.scalar.activation(e[:r, :], u[:r, :], EXP)
                nc.vector.tensor_scalar(out=xt[:r, :], in0=xt[:r, :], scalar1=0.0,
                                        scalar2=-1.0, op0=ALU.max, op1=ALU.add)
                nc.vector.tensor_tensor(out=xo[:r, :], in0=xt[:r, :], in1=e[:r, :],
                                        op=ALU.add)
                ptr = psum_t.tile([128, HC], bf, tag="ptr", name="ptr")
                for k in range(HC // 128):
                    nc.tensor.transpose(ptr[:, k * 128 : (k + 1) * 128],
                                        xo[:, k * 128 : (k + 1) * 128], idm[:, :])
                xT_sb = xTp.tile([128, HC], bf, tag="xT")
                nc.scalar.activation(xT_sb[:, :], ptr[:, :], CPY)
                return xT_sb
            else:
                xt3 = xt.rearrange("p (h c) -> p h c", h=H)
                hh = H // 2
                m1 = epip.tile([128, hh, C], f32, tag="m1")
                nc.vector.tensor_tensor(out=m1[:, :, :], in0=xt3[:, 0:hh, :],
                                        in1=xt3[:, hh : 2 * hh, :], op=ALU.add)
                m2 = epip.tile([128, C], f32, tag="m2")
                nc.vector.tensor_tensor(out=m2[:, :], in0=m1[:, 0, :], in1=m1[:, 1, :],
                                        op=ALU.add)
                for i in range(2, hh):
                    nc.vector.tensor_tensor(out=m2[:, :], in0=m2[:, :], in1=m1[:, i, :],
                                            op=ALU.add)
                ot = epip.tile([128, C], f32, tag="ot")
                nc.vector.scalar_tensor_tensor(
                    out=ot[:r, :], in0=m2[:r, :], scalar=1.0 / H, in1=lt2[:r, :],
                    op0=ALU.mult, op1=ALU.add,
                )
                nc.sync.dma_start(out=out_p[t * 128 : t * 128 + r, :], in_=ot[:r, :])
                return None

        # ------------------------------------------------------------------
        def maybe_ag(li, t):
            if t == S - 1:
                nc.gpsimd.collective_compute(
                    "AllGather", ALU.bypass,
                    replica_groups=[list(range(n_cores))],
                    ins=[pshard[li][0:rowsA, :].opt()],
                    outs=[pfullA[li].opt()],
                )
            elif t == T - 1:
                emit_bound(layers[li - 1])
                nc.gpsimd.collective_compute(
                    "AllGather", ALU.bypass,
                    replica_groups=[list(range(n_cores))],
                    ins=[pshard[li][rowsA:per_core, :].opt()],
                    outs=[pfullB[li].opt()],
                )

        def l1_lhsT_for(t):
            def get(k, kk):
                return xT1_sb[:, t * 128 : (t + 1) * 128]
            return get

        for t in range(T):
            dense_tile(layers[0], t, l1_lhsT_for(t))
            maybe_ag(1, t)

        for t in range(T):
            xT_sb = agg_tile(layers[0], t)
            dense_tile(layers[1], t, lambda k, kk, x=xT_sb: x[:, k * 128 : (k + 1) * 128])
            maybe_ag(2, t)
        for t in range(T):
            xT_sb = agg_tile(layers[1], t)
            dense_tile(layers[2], t, lambda k, kk, x=xT_sb: x[:, k * 128 : (k + 1) * 128])
            maybe_ag(3, t)
        for t in range(T):
            agg_tile(layers[2], t)

    nc.finalize()
    return nc


# --------------------------------------------------------------------------
# runner
# --------------------------------------------------------------------------

def _run(inputs, sim=False, trace=False, n_cores=N_CORES, tmpdir=None):
    in_maps, cfg, perm = _host_prep(inputs, n_cores)
    nc = _build(cfg)
    if sim:
        import concourse.bass_interp as bass_interp

        msim = bass_interp.MultiCoreSim(nc, n_cores)
        for c in range(n_cores):
            for k, v in in_maps[c].items():
                msim.cores[c].tensor(k)[:] = v
        msim.simulate(check_with_hw=True)
        outs = [np.array(msim.cores[c].mem_tensor("out")) for c in range(n_cores)]
        exec_ns = None
    else:
        from concourse.bass_utils import run_bass_kernel_spmd

        res = run_bass_kernel_spmd(
            nc, in_maps, list(range(n_cores)), trace=trace, tmpdir=tmpdir
        )
        outs = [res.results[c]["out"] for c in range(n_cores)]
        exec_ns = res.exec_time_ns
    out_new = np.concatenate(outs, 0)
    out = np.empty_like(out_new)
    out[...] = out_new[perm]
    return out.astype(np.float32), exec_ns


def kernel(**inputs) -> np.ndarray:
    out, _ = _run(inputs)
    return out


# revision 12
# speedup vs baseline: 1.0663x; 1.0663x over previous
"""3-layer GAT (PPI-style) forward on 8 Trainium2 NeuronCores — v4.

Strategy (SPMD, one NEFF on 8 cores):
  - Host: degree-balanced node permutation into 8 cores x 2500 nodes
    (tiles of 128 dst nodes), per-core tiles sorted by edge load; edges
    (WITHOUT self-loops) sorted by dst, chunked to 128-edge chunks with
    per-tile ragged chunk counts (padded even for fp8 DoubleRow pairs);
    int16 gather indices, fp8 one-hot scatter matrices in DoubleRow pair
    layout, bf16 transposed one-hots for the ed matmuls.
  - Payload AllGather split in two (tiles [0,S) / [S,T)) on separate
    Shared tensors; edges src-partitioned into the groups.
  - Dense phase per tile: [h | lin] = x @ [W | Wl] (bf16 PE); es/ed dots
    folded into the matmul; payload row [h (fp8) | es (f32)]; running
    per-core es-max tracked; after the dense loop a tiny AllGather + per-
    tile ed-max gives B_t,h = max(es_max + ed_max, 0) - DELTA, an upper
    bound on the attention logits used to shift exp() into fp8 range
    (shift cancels exactly in the softmax).
  - Aggregation per tile: ed via ohT^T @ edt; dma_gather per group;
    w8 = fp8(exp(leaky(es+ed) - B)); numerator AND denominator use the
    same w8 (quantization cancels in alpha): Gw8 = fp8(payload_h * w8),
    scatter-sum via fp8 DoubleRow matmuls (256-edge pairs, 0.5 cyc/col)
    accumulating [128,HC] + denominator [128,H] in PSUM; self-loop terms
    added in f32 in the epilogue (exact anchor for the denominator);
    normalize, skip+bias, ELU; PE-transpose feeds the next dense phase.
"""

import math
import numpy as np

N_CORES = 8
GROUP = 6          # max gather chunks per dma_gather call (even)
SPLIT_TILE = 16    # tiles [0, S) -> AG group A, [S, T) -> group B
DELTA = 4.0        # exp() headroom above the logit bound (fp8 range ctrl)


# --------------------------------------------------------------------------
# host-side prep
# --------------------------------------------------------------------------

def _balance_permutation(dst, n, n_cores, tiles_per_core, rows_last):
    import heapq

    deg = np.bincount(dst, minlength=n).astype(np.int64)
    order = np.argsort(-deg, kind="stable")
    n_tiles = n_cores * tiles_per_core
    caps = np.full(n_tiles, 128, np.int64)
    caps[tiles_per_core - 1 :: tiles_per_core] = rows_last
    heap = [(0, int(b)) for b in range(n_tiles)]
    heapq.heapify(heap)
    members = [[] for _ in range(n_tiles)]
    loads = np.zeros(n_tiles, np.int64)
    for node in order:
        while True:
            load, b = heapq.heappop(heap)
            if len(members[b]) < caps[b]:
                break
        members[b].append(node)
        loads[b] += deg[node]
        if len(members[b]) < caps[b]:
            heapq.heappush(heap, (int(loads[b]), b))

    perm_o2n = np.empty(n, np.int64)
    per_core = tiles_per_core * 128 - (128 - rows_last)
    for c in range(n_cores):
        bs = list(range(c * tiles_per_core, (c + 1) * tiles_per_core))
        full, short = bs[:-1], bs[-1]
        full.sort(key=lambda b: -loads[b])
        for t, b in enumerate(full + [short]):
            base = c * per_core + t * 128
            ids = np.asarray(members[b], np.int64)
            perm_o2n[ids] = base + np.arange(len(ids))
    return perm_o2n


def _wrap16_rep(a):
    w = a.reshape(-1, 16).T.astype(np.int16)
    return np.ascontiguousarray(np.tile(w, (8, 1)))


def _host_prep(inputs, n_cores=N_CORES):
    import ml_dtypes

    bf16 = ml_dtypes.bfloat16
    x = np.asarray(inputs["x"], np.float32)
    ei = np.asarray(inputs["edge_index"])
    n, f_in = x.shape
    src = ei[0].astype(np.int64)      # self-loops handled in the epilogue
    dst = ei[1].astype(np.int64)

    per_core = n // n_cores
    T = math.ceil(per_core / 128)
    rows_last = per_core - (T - 1) * 128
    S = SPLIT_TILE
    rowsA = S * 128
    rowsB = per_core - rowsA

    perm = _balance_permutation(dst, n, n_cores, T, rows_last)

    local = perm % per_core
    core = perm // per_core
    in_a = local < rowsA
    grow = np.where(in_a, core * rowsA + local, core * rowsB + (local - rowsA))

    dst_n = perm[dst]
    src_g = (~in_a[src]).astype(np.int64)      # 0 = group A, 1 = group B
    src_row = grow[src]

    core_of = dst_n // per_core
    loc_of = dst_n % per_core

    per_ctg = {}
    counts = np.zeros((n_cores, T, 2), np.int64)
    for c in range(n_cores):
        sel = core_of == c
        s, loc, g = src_row[sel], loc_of[sel], src_g[sel]
        o = np.argsort(loc, kind="stable")
        s, loc, g = s[o], loc[o], g[o]
        tile_of = loc // 128
        for t in range(T):
            mt = tile_of == t
            for gg in range(2):
                m = mt & (g == gg)
                per_ctg[c, t, gg] = (s[m], loc[m] - t * 128)
                counts[c, t, gg] = m.sum()

    # per-tile ragged chunk counts (even, for DoubleRow pairs)
    def even_ceil(v):
        k = max(2, math.ceil(v / 128))
        return k + (k & 1)

    nchunks = [(even_ceil(counts[:, t, 0].max()), even_ceil(counts[:, t, 1].max()))
               for t in range(T)]
    ntot = [a + b for a, b in nchunks]

    co_idx = np.concatenate([[0], np.cumsum([v * 8 for v in ntot])])
    co_oh = np.concatenate([[0], np.cumsum([v * 128 for v in ntot])])

    src16 = np.zeros((n_cores, 128, int(co_idx[-1])), np.int16)
    oh8 = np.zeros((n_cores, 128, int(co_oh[-1])), np.uint8)   # fp8e4m3 bits
    ohTs = np.zeros((n_cores, 128, int(co_oh[-1])), bf16)
    ONE8 = 0x38                                                # 1.0 in e4m3
    pp = np.arange(128)
    for c in range(n_cores):
        for t in range(T):
            nA, nB = nchunks[t]
            cap = (nA + nB) * 128
            ps = np.zeros(cap, np.int64)
            pl = np.full(cap, -1, np.int64)
            for gg, off, ncg in ((0, 0, nA), (1, nA * 128, nB)):
                s_, l_ = per_ctg[c, t, gg]
                e = len(s_)
                ps[off : off + e] = s_
                pl[off : off + e] = l_
            src16[c, :, co_idx[t] : co_idx[t + 1]] = _wrap16_rep(ps)
            dl = pl.reshape(nA + nB, 128)
            for cch in range(nA + nB):
                m = dl[cch] >= 0
                j = dl[cch, m]
                # DoubleRow pair layout: pair p = cch//2, sub i = cch%2,
                # cols [p*256 + i*128 + j]
                base = co_oh[t] + (cch // 2) * 256 + (cch % 2) * 128
                oh8[c, pp[m], base + j] = ONE8
                ohTs[c, j, co_oh[t] + cch * 128 + pp[m]] = 1

    rows_pad = T * 128
    x_perm = np.zeros((n, f_in), np.float32)
    x_perm[perm] = x
    xT = []
    for c in range(n_cores):
        blk = np.zeros((rows_pad, f_in), np.float32)
        blk[:per_core] = x_perm[c * per_core : (c + 1) * per_core]
        xT.append(np.ascontiguousarray(blk.T).astype(bf16))

    g = lambda k: np.asarray(inputs[k], np.float32)
    h1, c1 = g("a1s").shape
    h3, c3 = g("a3s").shape
    d1 = h1 * c1

    def fold(Wk, ak_s, ak_d, h, cc):
        W = g(Wk)
        a_s, a_d = g(ak_s), g(ak_d)
        waS = np.stack([W[:, i * cc : (i + 1) * cc] @ a_s[i] for i in range(h)], 1)
        waD = np.stack([W[:, i * cc : (i + 1) * cc] @ a_d[i] for i in range(h)], 1)
        return np.concatenate([waS, waD], 1).astype(bf16)  # [din, 2h]

    waug1 = np.concatenate([g("W1"), g("Wl1")], 1).astype(bf16)
    waug2 = np.concatenate([g("W2"), g("Wl2")], 1).astype(bf16)
    waug3 = np.concatenate([g("W3"), g("Wl3")], 1).astype(bf16)
    wsd1 = fold("W1", "a1s", "a1d", h1, c1)
    wsd2 = fold("W2", "a2s", "a2d", h1, c1)
    wsd3 = fold("W3", "a3s", "a3d", h3, c3)

    rep = lambda v: np.ascontiguousarray(
        np.broadcast_to(v[None, :], (128, v.shape[0]))
    ).astype(np.float32)
    base = dict(
        waug1=waug1, waug2=waug2, waug3=waug3,
        wsd1=wsd1, wsd2=wsd2, wsd3=wsd3,
        bsum1=rep(g("b1") + g("bl1")),
        bsum2=rep(g("b2") + g("bl2")),
        bsum3=rep(g("b3") + g("bl3")),
        idmat=np.eye(128, dtype=bf16),
    )
    in_maps = []
    for c in range(n_cores):
        m = dict(base)
        m["xT1"] = xT[c]
        m["src16"] = src16[c]
        m["oh8"] = np.ascontiguousarray(oh8[c])
        m["ohT"] = np.ascontiguousarray(ohTs[c])
        in_maps.append(m)

    cfg = dict(
        n=n, f_in=f_in, n_cores=n_cores, per_core=per_core,
        tiles_per_core=T, rows_last=rows_last, rows_pad=rows_pad,
        s_tile=S, rows_a=rowsA, rows_b=rowsB,
        nchunks=nchunks, co_idx=[int(v) for v in co_idx],
        co_oh=[int(v) for v in co_oh],
        h1=h1, c1=c1, d1=d1, h3=h3, c3=c3,
    )
    return in_maps, cfg, perm


# --------------------------------------------------------------------------
# bass program
# --------------------------------------------------------------------------

def _layer_dims(cfg):
    """Payload row: [h fp8 (HC bytes, 4-pad)] [es f32 (4H bytes)], row size
    a 256B multiple (dma_gather restriction)."""
    out = []
    for li in (1, 2, 3):
        if li < 3:
            h, c = cfg["h1"], cfg["c1"]
            din = cfg["f_in"] if li == 1 else cfg["d1"]
            nlin = cfg["d1"]
        else:
            h, c = cfg["h3"], cfg["c3"]
            din = cfg["d1"]
            nlin = cfg["c3"]
        hc = h * c
        esb = math.ceil(hc / 4) * 4        # byte offset of es
        pw = math.ceil((esb + 4 * h) / 256) * 256
        kch = math.ceil(din / 128)
        out.append(dict(li=li, din=din, kch=kch, hc=hc, nlin=nlin,
                        h=h, c=c, es4=esb // 4, pw=pw, naug=hc + nlin))
    return out


def _groups(n, grp):
    out = []
    o = 0
    while o < n:
        out.append((o, min(grp, n - o)))
        o += grp
    return out


def _build(cfg):
    import concourse.bass as bass
    import concourse.bacc as bacc
    import concourse.mybir as mybir
    import concourse.tile as tile
    from contextlib import ExitStack

    f32 = mybir.dt.float32
    bf = mybir.dt.bfloat16
    i16 = mybir.dt.int16
    u8 = mybir.dt.uint8
    f8 = mybir.dt.float8e4
    EXP = mybir.ActivationFunctionType.Exp
    CPY = mybir.ActivationFunctionType.Copy
    ALU = mybir.AluOpType
    DR = mybir.MatmulPerfMode.DoubleRow

    n_cores = cfg["n_cores"]
    n = cfg["n"]
    T = cfg["tiles_per_core"]
    rows_last = cfg["rows_last"]
    per_core = cfg["per_core"]
    S = cfg["s_tile"]
    rowsA, rowsB = cfg["rows_a"], cfg["rows_b"]
    NCH = cfg["nchunks"]
    CO_IDX = cfg["co_idx"]
    CO_OH = cfg["co_oh"]
    NMAX = max(a + b for a, b in NCH)
    GRP = GROUP
    layers = _layer_dims(cfg)

    nc = bacc.Bacc(None, target_bir_lowering=False, num_swdge_queues=2)

    # ---- parameters -----------------------------------------------------
    xT1 = nc.declare_dram_parameter("xT1", [cfg["f_in"], T * 128], bf, isOutput=False)
    waug_p, wsd_p, bsum_p = {}, {}, {}
    for L in layers:
        li = L["li"]
        waug_p[li] = nc.declare_dram_parameter(
            f"waug{li}", [L["din"], L["naug"]], bf, isOutput=False)
        wsd_p[li] = nc.declare_dram_parameter(
            f"wsd{li}", [L["din"], 2 * L["h"]], bf, isOutput=False)
        bsum_p[li] = nc.declare_dram_parameter(
            f"bsum{li}", [128, L["nlin"]], f32, isOutput=False)
    src16_p = nc.declare_dram_parameter("src16", [128, CO_IDX[-1]], i16, isOutput=False)
    oh_p = nc.declare_dram_parameter("oh8", [128, CO_OH[-1]], u8, isOutput=False)
    ohT_p = nc.declare_dram_parameter("ohT", [128, CO_OH[-1]], bf, isOutput=False)
    id_p = nc.declare_dram_parameter("idmat", [128, 128], bf, isOutput=False)
    out_p = nc.declare_dram_parameter("out", [per_core, cfg["c3"]], f32, isOutput=True)

    with tile.TileContext(nc, num_cores=n_cores) as tc, ExitStack() as ctx:
        # ---- dram scratch ----------------------------------------------
        dram = ctx.enter_context(tc.tile_pool(name="dram", bufs=1, space="DRAM"))
        pshard = {L["li"]: dram.tile([per_core, L["pw"]], u8, tag=f"pshard{L['li']}",
                                     name=f"pshard{L['li']}") for L in layers}
        pfullA = {L["li"]: dram.tile([n_cores * rowsA, L["pw"]], u8,
                                     tag=f"pfa{L['li']}", name=f"pfa{L['li']}",
                                     addr_space="Shared") for L in layers}
        pfullB = {L["li"]: dram.tile([n_cores * rowsB, L["pw"]], u8,
                                     tag=f"pfb{L['li']}", name=f"pfb{L['li']}",
                                     addr_space="Shared") for L in layers}
        linb = {L["li"]: dram.tile([T * 128, L["nlin"]], f32, tag=f"lin{L['li']}",
                                   name=f"lin{L['li']}") for L in layers}
        esd = {L["li"]: dram.tile([1, L["h"]], f32, tag=f"esd{L['li']}",
                                  name=f"esd{L['li']}") for L in layers}
        esg = {L["li"]: dram.tile([n_cores, L["h"]], f32, tag=f"esg{L['li']}",
                                  name=f"esg{L['li']}", addr_space="Shared")
               for L in layers}

        # ---- pools ------------------------------------------------------
        consts = ctx.enter_context(tc.tile_pool(name="consts", bufs=1))
        wtp = ctx.enter_context(tc.tile_pool(name="wtp", bufs=1))
        xTp = ctx.enter_context(tc.tile_pool(name="xTp", bufs=2))
        ptp = ctx.enter_context(tc.tile_pool(name="ptp", bufs=2))
        ltp = ctx.enter_context(tc.tile_pool(name="ltp", bufs=2))
        gp = ctx.enter_context(tc.tile_pool(name="gp", bufs=3))
        gwp = ctx.enter_context(tc.tile_pool(name="gwp", bufs=2))
        idxp = ctx.enter_context(tc.tile_pool(name="idxp", bufs=2))
        ohp = ctx.enter_context(tc.tile_pool(name="ohp", bufs=2))
        lgp = ctx.enter_context(tc.tile_pool(name="lgp", bufs=4))
        epip = ctx.enter_context(tc.tile_pool(name="epip", bufs=1))
        recp = ctx.enter_context(tc.tile_pool(name="recp", bufs=4))
        psum_d = ctx.enter_context(tc.tile_pool(name="psum_d", bufs=1, space="PSUM"))
        psum_a = ctx.enter_context(tc.tile_pool(name="psum_a", bufs=1, space="PSUM"))
        psum_e = ctx.enter_context(tc.tile_pool(name="psum_e", bufs=2, space="PSUM"))
        psum_n = ctx.enter_context(tc.tile_pool(name="psum_n", bufs=1, space="PSUM"))
        psum_t = ctx.enter_context(tc.tile_pool(name="psum_t", bufs=1, space="PSUM"))

        # ---- constants ---------------------------------------------------
        idm = consts.tile([128, 128], bf, tag="idm")
        nc.sync.dma_start(out=idm[:, :], in_=id_p[:, :])
        xT1_sb = consts.tile([cfg["f_in"], T * 128], bf, tag="xT1")
        nc.sync.dma_start(out=xT1_sb[:, :], in_=xT1[:, :])
        wt, wsd, bsum, edts, esmx, bneg = {}, {}, {}, {}, {}, {}
        for L in layers:
            li, KCH, DIN, H = L["li"], L["kch"], L["din"], L["h"]
            for k in range(KCH):
                kk = min(128, DIN - k * 128)
                w = wtp.tile([128, L["naug"]], bf, tag=f"w{li}_{k}", name=f"w{li}_{k}")
                nc.sync.dma_start(out=w[:kk, :], in_=waug_p[li][k * 128 : k * 128 + kk, :])
                wt[li, k] = w
                s = wtp.tile([128, 2 * H], bf, tag=f"s{li}_{k}", name=f"s{li}_{k}")
                nc.sync.dma_start(out=s[:kk, :], in_=wsd_p[li][k * 128 : k * 128 + kk, :])
                wsd[li, k] = s
            b = consts.tile([128, L["nlin"]], f32, tag=f"b{li}", name=f"b{li}")
            nc.sync.dma_start(out=b[:, :], in_=bsum_p[li][:, :])
            bsum[li] = b
            e = consts.tile([128, T * H], bf, tag=f"e{li}", name=f"e{li}")
            edts[li] = e
            em = consts.tile([128, H], f32, tag=f"esm{li}", name=f"esm{li}")
            esmx[li] = em
            bn = consts.tile([128, T * H], f32, tag=f"bn{li}", name=f"bn{li}")
            bneg[li] = bn

        qn = [0]

        def rows_of(t):
            return 128 if t < T - 1 else rows_last

        def chunks(w):
            return [(c0, min(c0 + 512, w)) for c0 in range(0, w, 512)]

        # ---------------- dense phase for one tile -----------------------
        def dense_tile(L, t, get_lhsT):
            li, DIN, KCH = L["li"], L["din"], L["kch"]
            H, C, HC = L["h"], L["c"], L["hc"]
            NLIN, ES4, PW = L["nlin"], L["es4"], L["pw"]
            r = rows_of(t)

            lhsTs = {}

            def lhsT_of(k, kk):
                if k not in lhsTs:
                    lhsTs[k] = get_lhsT(k, kk)
                return lhsTs[k]

            wA = HC + NLIN if li == 3 else HC
            pse = psum_e.tile([128, 2 * H], f32, tag="pe", name="pse")
            pdA = psum_d.tile([128, 1024], f32, tag="pd", name="pdA")
            for k in range(KCH):
                kk = min(128, DIN - k * 128)
                lhsT = lhsT_of(k, kk)
                st, sp = (k == 0), (k == KCH - 1)
                for c0, c1 in chunks(wA):
                    nc.tensor.matmul(pdA[:, c0:c1], lhsT[:kk, :],
                                     wt[li, k][:kk, c0:c1], start=st, stop=sp)
                nc.tensor.matmul(pse[:, :], lhsT[:kk, :], wsd[li, k][:kk, :],
                                 start=st, stop=sp)

            # payload assembly (h in fp8, es in f32)
            pt = ptp.tile([128, PW], u8, tag="pt")
            ptb = pt.bitcast(f8)
            nc.scalar.activation(ptb[:, :HC], pdA[:, :HC], CPY)
            ptf = pt.bitcast(f32)
            nc.scalar.activation(ptf[:, ES4 : ES4 + H], pse[:, :H], CPY)
            nc.scalar.activation(edts[li][:, t * H : (t + 1) * H], pse[:, H : 2 * H], CPY)
            # track per-core es max
            if t == 0:
                nc.vector.tensor_copy(esmx[li][:, :], pse[:, :H])
            else:
                nc.vector.tensor_tensor(out=esmx[li][:, :], in0=esmx[li][:, :],
                                        in1=pse[:, :H], op=ALU.max)
            nc.sync.dma_start(out=pshard[li][t * 128 : t * 128 + r, :], in_=pt[:r, :])
            if li < 3:
                pdB = psum_d.tile([128, 1024], f32, tag="pd", name="pdB")
                for k in range(KCH):
                    kk = min(128, DIN - k * 128)
                    lhsT = lhsT_of(k, kk)
                    st, sp = (k == 0), (k == KCH - 1)
                    for c0, c1 in chunks(NLIN):
                        nc.tensor.matmul(pdB[:, c0:c1], lhsT[:kk, :],
                                         wt[li, k][:kk, HC + c0 : HC + c1],
                                         start=st, stop=sp)
            else:
                pdB = pdA
            loff = HC if li == 3 else 0
            lt = ltp.tile([128, NLIN], f32, tag="lt")
            nc.vector.tensor_tensor(out=lt[:, :], in0=pdB[:, loff : loff + NLIN],
                                    in1=bsum[li][:, :], op=ALU.add)
            nc.sync.dma_start(out=linb[li][t * 128 : t * 128 + r, :], in_=lt[:r, :])

        # ------------- per-layer logit bound B (after dense loop) ---------
        def emit_bound(L):
            li, H = L["li"], L["h"]
            esr = consts.tile([128, H], f32, tag=f"esr{li}", name=f"esr{li}")
            nc.gpsimd.partition_all_reduce(esr[:, :], esmx[li][:, :], 128,
                                           reduce_op=bass.bass_isa.ReduceOp.max)
            nc.sync.dma_start(out=esd[li][:, :], in_=esr[0:1, :])
            nc.gpsimd.collective_compute(
                "AllGather", ALU.bypass,
                replica_groups=[list(range(n_cores))],
                ins=[esd[li].opt()], outs=[esg[li].opt()],
            )
            ess = consts.tile([n_cores, H], f32, tag=f"ess{li}", name=f"ess{li}")
            nc.sync.dma_start(out=ess[:, :], in_=esg[li][:, :])
            essr = consts.tile([n_cores, H], f32, tag=f"essr{li}", name=f"essr{li}")
            nc.gpsimd.partition_all_reduce(essr[:, :], ess[:, :], n_cores,
                                           reduce_op=bass.bass_isa.ReduceOp.max)
            esb_t = consts.tile([128, H], f32, tag=f"esb{li}", name=f"esb{li}")
            nc.gpsimd.partition_broadcast(esb_t[:, :], essr[0:1, :])
            edm = consts.tile([128, T * H], bf, tag=f"edm{li}", name=f"edm{li}")
            nc.gpsimd.partition_all_reduce(edm[:, :], edts[li][:, :], 128,
                                           reduce_op=bass.bass_isa.ReduceOp.max)
            bn = bneg[li]
            nc.vector.tensor_tensor(
                out=bn.rearrange("p (t h) -> p t h", h=H),
                in0=edm.rearrange("p (t h) -> p t h", h=H),
                in1=esb_t.unsqueeze(1).broadcast_to([128, T, H]), op=ALU.add)
            nc.vector.tensor_scalar(out=bn[:, :], in0=bn[:, :], scalar1=0.0,
                                    scalar2=None, op0=ALU.max)
            nc.vector.tensor_scalar(out=bn[:, :], in0=bn[:, :], scalar1=-1.0,
                                    scalar2=DELTA, op0=ALU.mult, op1=ALU.add)

        # ---------------- aggregation for one tile ------------------------
        def agg_tile(L, t):
            li = L["li"]
            H, C, HC = L["h"], L["c"], L["hc"]
            NLIN, ES4, PW = L["nlin"], L["es4"], L["pw"]
            r = rows_of(t)
            nA, nB = NCH[t]
            NT = nA + nB
            NPAIR = NT // 2

            s16 = idxp.tile([128, NMAX * 8], i16, tag="s16")
            nc.sync.dma_start(out=s16[:, : NT * 8],
                              in_=src16_p[:, CO_IDX[t] : CO_IDX[t + 1]])
            oh_sb = ohp.tile([128, NMAX * 128], u8, tag="oh")
            nc.sync.dma_start(out=oh_sb[:, : NT * 128],
                              in_=oh_p[:, CO_OH[t] : CO_OH[t + 1]])
            oh8 = oh_sb.bitcast(f8)
            ohT_sb = ohp.tile([128, NMAX * 128], bf, tag="ohT")
            nc.sync.dma_start(out=ohT_sb[:, : NT * 128],
                              in_=ohT_p[:, CO_OH[t] : CO_OH[t + 1]])

            pe = psum_e.tile([128, NMAX * H], f32, tag="pe", name="pe")
            for cch in range(NT):
                nc.tensor.matmul(pe[:, cch * H : (cch + 1) * H],
                                 ohT_sb[:, cch * 128 : (cch + 1) * 128],
                                 edts[li][:, t * H : (t + 1) * H],
                                 start=True, stop=True)

            pa = psum_a.tile([128, HC], f32, tag="pa", name="pa")
            den = psum_n.tile([128, H], f32, tag="den", name="den")
            bnB = bneg[li][:, t * H : (t + 1) * H]
            glist = [(pfullA[li], o, sz) for o, sz in _groups(nA, GRP)]
            glist += [(pfullB[li], nA + o, sz) for o, sz in _groups(nB, GRP)]
            for src_t, goff, gsz in glist:
                G = gp.tile([128, GRP, PW], u8, tag="G")
                nc.gpsimd.dma_gather(
                    out_ap=G[:, :gsz, :],
                    in_ap=src_t[:, :],
                    idxs_ap=s16[:, goff * 8 : (goff + gsz) * 8],
                    num_idxs=gsz * 128,
                    num_idxs_reg=gsz * 128,
                    elem_size=PW,
                    queue_num=qn[0] % 2,
                )
                qn[0] += 1
                Gf = G.bitcast(f32)
                tl = lgp.tile([128, GRP, H], f32, tag="tl")
                wf = lgp.tile([128, GRP, H], f32, tag="wf")
                nc.vector.tensor_tensor(
                    out=tl[:, :gsz, :], in0=Gf[:, :gsz, ES4 : ES4 + H],
                    in1=pe[:, goff * H : (goff + gsz) * H].rearrange(
                        "p (g h) -> p g h", h=H),
                    op=ALU.add,
                )
                nc.vector.scalar_tensor_tensor(
                    out=wf[:, :gsz, :], in0=tl[:, :gsz, :], scalar=0.2,
                    in1=tl[:, :gsz, :], op0=ALU.mult, op1=ALU.max,
                )
                nc.vector.tensor_tensor(
                    out=wf[:, :gsz, :], in0=wf[:, :gsz, :],
                    in1=bnB.unsqueeze(1).broadcast_to([128, gsz, H]), op=ALU.add)
                web8 = lgp.tile([128, GRP, H], f8, tag="web8")
                nc.scalar.activation(web8[:, :gsz, :], wf[:, :gsz, :], EXP)
                if li == 1:
                    # L1: weight the payload on DVE, shared one-hot lhsT
                    G8 = G.bitcast(f8)[:, :gsz, :HC].rearrange(
                        "p g (h c) -> p g h c", h=H)
                    webB = web8[:, :gsz].unsqueeze(3).broadcast_to(
                        [128, gsz, H, C])
                    Gw8 = gwp.tile([128, GRP, HC], f8, tag="Gw8")
                    nc.vector.tensor_tensor(
                        out=Gw8[:, :gsz].rearrange("p g (h c) -> p g h c", h=H),
                        in0=G8, in1=webB, op=ALU.mult)
                    for c2 in range(gsz // 2):
                        p = goff // 2 + c2
                        lhsT = oh8[:, p * 256 : (p + 1) * 256].rearrange(
                            "q (two m) -> q two m", two=2)
                        st, sp = (p == 0), (p == NPAIR - 1)
                        for c0, c1 in chunks(HC):
                            nc.tensor.matmul(
                                pa[:, c0:c1], lhsT,
                                Gw8[:, 2 * c2 : 2 * c2 + 2, c0:c1],
                                start=st, stop=sp, perf_mode=DR)
                        nc.tensor.matmul(den[:, :], lhsT,
                                         web8[:, 2 * c2 : 2 * c2 + 2, :],
                                         start=st, stop=sp, perf_mode=DR)
                else:
                    # L2/L3: weight the ONE-HOT instead (half the DVE work;
                    # w*h product then accumulates in f32 on the PE).
                    # Pair 0 uses the weighted-payload form so the PSUM
                    # start= flags cover whole 512-col banks (sub-bank
                    # starts wipe neighbouring heads' accumulation).
                    G8 = G.bitcast(f8)
                    for c2 in range(gsz // 2):
                        p = goff // 2 + c2
                        ohpair = oh8[:, p * 256 : (p + 1) * 256].rearrange(
                            "q (two m) -> q two m", two=2)
                        st, sp = (p == 0), (p == NPAIR - 1)
                        if p == 0:
                            G8v = G.bitcast(f8)[:, 0:2, :HC].rearrange(
                                "p g (h c) -> p g h c", h=H)
                            webB = web8[:, 0:2].unsqueeze(3).broadcast_to(
                                [128, 2, H, C])
                            Gw0 = gwp.tile([128, 2, HC], f8, tag="Gw0")
                            nc.vector.tensor_tensor(
                                out=Gw0.rearrange("p g (h c) -> p g h c", h=H),
                                in0=G8v, in1=webB, op=ALU.mult)
                            for c0, c1 in chunks(HC):
                                nc.tensor.matmul(
                                    pa[:, c0:c1], ohpair, Gw0[:, :, c0:c1],
                                    start=True, stop=sp, perf_mode=DR)
                        else:
                            ohw = gwp.tile([128, H, 2, 128], f8, tag="ohw",
                                           bufs=4)
                            nc.vector.tensor_tensor(
                                out=ohw[:, :, :, :],
                                in0=ohpair.unsqueeze(1).broadcast_to(
                                    [128, H, 2, 128]),
                                in1=web8[:, 2 * c2 : 2 * c2 + 2, :]
                                    .rearrange("p two h -> p h two").unsqueeze(3)
                                    .broadcast_to([128, H, 2, 128]),
                                op=ALU.mult)
                            for h in range(H):
                                nc.tensor.matmul(
                                    pa[:, h * C : (h + 1) * C], ohw[:, h, :, :],
                                    G8[:, 2 * c2 : 2 * c2 + 2,
                                       h * C : (h + 1) * C],
                                    start=False, stop=sp, perf_mode=DR)
                        nc.tensor.matmul(den[:, :], ohpair,
                                         web8[:, 2 * c2 : 2 * c2 + 2, :],
                                         start=st, stop=sp, perf_mode=DR)

            # ---- epilogue ----
            pac = epip.tile([128, HC], f32, tag="pac", bufs=2)
            nc.scalar.activation(pac[:, :], pa[:, :], CPY)
            dnc = recp.tile([128, H], f32, tag="dnc")
            nc.scalar.activation(dnc[:, :], den[:, :], CPY)
            # self-loop terms in f32 (exact anchor)
            pself = ptp.tile([128, PW], u8, tag="pself")
            nc.sync.dma_start(out=pself[:r, :], in_=pshard[li][t * 128 : t * 128 + r, :])
            ps8 = pself.bitcast(f8)
            psf = pself.bitcast(f32)
            tls = recp.tile([128, H], f32, tag="tls")
            nc.vector.tensor_tensor(out=tls[:r, :], in0=psf[:r, ES4 : ES4 + H],
                                    in1=edts[li][:r, t * H : (t + 1) * H], op=ALU.add)
            nc.vector.scalar_tensor_tensor(out=tls[:r, :], in0=tls[:r, :], scalar=0.2,
                                           in1=tls[:r, :], op0=ALU.mult, op1=ALU.max)
            nc.vector.tensor_tensor(out=tls[:r, :], in0=tls[:r, :], in1=bnB[:r, :],
                                    op=ALU.add)
            ws = recp.tile([128, H], f32, tag="ws")
            nc.scalar.activation(ws[:r, :], tls[:r, :], EXP)
            dent = recp.tile([128, H], f32, tag="dent")
            nc.vector.tensor_tensor(out=dent[:r, :], in0=dnc[:r, :], in1=ws[:r, :],
                                    op=ALU.add)
            nc.vector.tensor_scalar(out=dent[:, :], in0=dent[:, :], scalar1=1e-30,
                                    scalar2=None, op0=ALU.max)
            rec = recp.tile([128, H], f32, tag="rec")
            nc.vector.reciprocal(rec[:, :], dent[:, :])
            xt = epip.tile([128, HC], f32, tag="xt", bufs=2)
            for h in range(H):
                nc.vector.scalar_tensor_tensor(
                    out=xt[:r, h * C : (h + 1) * C], in0=ps8[:r, h * C : (h + 1) * C],
                    scalar=ws[:r, h : h + 1], in1=pac[:r, h * C : (h + 1) * C],
                    op0=ALU.mult, op1=ALU.add)
                nc.vector.tensor_scalar(
                    out=xt[:, h * C : (h + 1) * C], in0=xt[:, h * C : (h + 1) * C],
                    scalar1=rec[:, h : h + 1], scalar2=None, op0=ALU.mult,
                )
            lt2 = ltp.tile([128, NLIN], f32, tag="lt2")
            nc.sync.dma_start(out=lt2[:r, :], in_=linb[li][t * 128 : t * 128 + r, :])
            if li < 3:
                u = epip.tile([128, HC], f32, tag="u")
                e = epip.tile([128, HC], f32, tag="e")
                xo = epip.tile([128, HC], bf, tag="xo")
                if r < 128:
                    nc.vector.memset(xo[:, :], 0)
                nc.vector.tensor_tensor(out=xt[:r, :], in0=xt[:r, :], in1=lt2[:r, :],
                                        op=ALU.add)
                nc.vector.tensor_scalar(out=u[:r, :], in0=xt[:r, :], scalar1=0.0,
                                        scalar2=None, op0=ALU.min)
                nc.scalar.activation(e[:r, :], u[:r, :], EXP)
                nc.vector.tensor_scalar(out=xt[:r, :], in0=xt[:r, :], scalar1=0.0,
                                        scalar2=-1.0, op0=ALU.max, op1=ALU.add)
                nc.vector.tensor_tensor(out=xo[:r, :], in0=xt[:r, :], in1=e[:r, :],
                                        op=ALU.add)
                ptr = psum_t.tile([128, HC], bf, tag="ptr", name="ptr")
                for k in range(HC // 128):
                    nc.tensor.transpose(ptr[:, k * 128 : (k + 1) * 128],
                                        xo[:, k * 128 : (k + 1) * 128], idm[:, :])
                xT_sb = xTp.tile([128, HC], bf, tag="xT")
                nc.scalar.activation(xT_sb[:, :], ptr[:, :], CPY)
                return xT_sb
            else:
                xt3 = xt.rearrange("p (h c) -> p h c", h=H)
                hh = H // 2
                m1 = epip.tile([128, hh, C], f32, tag="m1")
                nc.vector.tensor_tensor(out=m1[:, :, :], in0=xt3[:, 0:hh, :],
                                        in1=xt3[:, hh : 2 * hh, :], op=ALU.add)
                m2 = epip.tile([128, C], f32, tag="m2")
                nc.vector.tensor_tensor(out=m2[:, :], in0=m1[:, 0, :], in1=m1[:, 1, :],
                                        op=ALU.add)
                for i in range(2, hh):
                    nc.vector.tensor_tensor(out=m2[:, :], in0=m2[:, :], in1=m1[:, i, :],
                                            op=ALU.add)
                ot = epip.tile([128, C], f32, tag="ot")
                nc.vector.scalar_tensor_tensor(
                    out=ot[:r, :], in0=m2[:r, :], scalar=1.0 / H, in1=lt2[:r, :],
                    op0=ALU.mult, op1=ALU.add,
                )
                nc.sync.dma_start(out=out_p[t * 128 : t * 128 + r, :], in_=ot[:r, :])
                return None

        # ------------------------------------------------------------------
        def maybe_ag(li, t):
            if t == S - 1:
                nc.gpsimd.collective_compute(
                    "AllGather", ALU.bypass,
                    replica_groups=[list(range(n_cores))],
                    ins=[pshard[li][0:rowsA, :].opt()],
                    outs=[pfullA[li].opt()],
                )
            elif t == T - 1:
                emit_bound(layers[li - 1])
                nc.gpsimd.collective_compute(
                    "AllGather", ALU.bypass,
                    replica_groups=[list(range(n_cores))],
                    ins=[pshard[li][rowsA:per_core, :].opt()],
                    outs=[pfullB[li].opt()],
                )

        def l1_lhsT_for(t):
            def get(k, kk):
                return xT1_sb[:, t * 128 : (t + 1) * 128]
            return get

        for t in range(T):
            dense_tile(layers[0], t, l1_lhsT_for(t))
            maybe_ag(1, t)

        for t in range(T):
            xT_sb = agg_tile(layers[0], t)
            dense_tile(layers[1], t, lambda k, kk, x=xT_sb: x[:, k * 128 : (k + 1) * 128])
            maybe_ag(2, t)
        for t in range(T):
            xT_sb = agg_tile(layers[1], t)
            dense_tile(layers[2], t, lambda k, kk, x=xT_sb: x[:, k * 128 : (k + 1) * 128])
            maybe_ag(3, t)
        for t in range(T):
            agg_tile(layers[2], t)

    nc.finalize()
    return nc


# --------------------------------------------------------------------------
# runner
# --------------------------------------------------------------------------

def _run(inputs, sim=False, trace=False, n_cores=N_CORES, tmpdir=None):
    in_maps, cfg, perm = _host_prep(inputs, n_cores)
    nc = _build(cfg)
    if sim:
        import concourse.bass_interp as bass_interp

        msim = bass_interp.MultiCoreSim(nc, n_cores)
        for c in range(n_cores):
            for k, v in in_maps[c].items():
                msim.cores[c].tensor(k)[:] = v
        msim.simulate(check_with_hw=True)
        outs = [np.array(msim.cores[c].mem_tensor("out")) for c in range(n_cores)]
        exec_ns = None
    else:
        from concourse.bass_utils import run_bass_kernel_spmd

        res = run_bass_kernel_spmd(
            nc, in_maps, list(range(n_cores)), trace=trace, tmpdir=tmpdir
        )
        outs = [res.results[c]["out"] for c in range(n_cores)]
        exec_ns = res.exec_time_ns
    out_new = np.concatenate(outs, 0)
    out = np.empty_like(out_new)
    out[...] = out_new[perm]
    return out.astype(np.float32), exec_ns


def kernel(**inputs) -> np.ndarray:
    out, _ = _run(inputs)
    return out


# revision 38
# speedup vs baseline: 1.0677x; 1.0013x over previous
"""3-layer GAT (PPI-style) forward on 8 Trainium2 NeuronCores — v3.

Strategy (SPMD, one NEFF on 8 cores):
  - Host: add self-loops, degree-balanced node permutation into 8 cores x
    2500 nodes (tiles of 128 dst nodes); per-core tiles sorted by edge load
    so that the per-tile-index max across cores is minimal; edges sorted by
    dst, chunked to 128-edge chunks with PER-TILE ragged chunk counts;
    int16 gather-index arrays and STATIC one-hot scatter matrices
    precomputed (flattened ragged layout).
  - Payload AllGather is SPLIT in two (tiles [0,S) and [S,T)) on separate
    Shared tensors so AG-A overlaps the tail of the producing loop; edges
    are src-partitioned into the two groups with separate gathers.
  - Dense phase per tile: [h | lin] = x @ [W | Wl] (bf16, PE); es/ed
    attention dots folded into the matmul as extra columns; payload row
    [h0|1|h1|1|...|es] (fp8 + f32 es tail) staged to DRAM shard.
  - Aggregation per tile: ed via tiny matmuls ohT^T @ edt; payload
    dma_gather per group; w = exp(leaky(es+ed)) scales the payload with a
    broadcast multiply; scatter-sum via matmul with the static one-hot
    lhsT (512-col chunks, PSUM-accumulated); PSUM drained early via scalar
    copy; normalize, add skip+bias, ELU; PE-transpose keeps the next
    layer's lhsT in SBUF; next layer's dense phase inlined per tile.
"""

import math
import numpy as np

N_CORES = 8
GROUP = 6          # max gather chunks per dma_gather call
SPLIT_TILE = 16    # tiles [0, S) -> AG group A, [S, T) -> group B


# --------------------------------------------------------------------------
# host-side prep (data layout / graph partitioning / static one-hots)
# --------------------------------------------------------------------------

def _balance_permutation(dst, n, n_cores, tiles_per_core, rows_last):
    """Greedy balance: nodes -> 128-row dst tiles with ~equal edge counts.
    Within each core, full tiles are then ordered by load (desc) so the
    max-over-cores load per tile index stays near the mean."""
    import heapq

    deg = np.bincount(dst, minlength=n).astype(np.int64)
    order = np.argsort(-deg, kind="stable")
    n_tiles = n_cores * tiles_per_core
    caps = np.full(n_tiles, 128, np.int64)
    caps[tiles_per_core - 1 :: tiles_per_core] = rows_last
    heap = [(0, int(b)) for b in range(n_tiles)]
    heapq.heapify(heap)
    members = [[] for _ in range(n_tiles)]
    loads = np.zeros(n_tiles, np.int64)
    for node in order:
        while True:
            load, b = heapq.heappop(heap)
            if len(members[b]) < caps[b]:
                break
        members[b].append(node)
        loads[b] += deg[node]
        if len(members[b]) < caps[b]:
            heapq.heappush(heap, (int(loads[b]), b))

    perm_o2n = np.empty(n, np.int64)
    per_core = tiles_per_core * 128 - (128 - rows_last)
    for c in range(n_cores):
        bs = list(range(c * tiles_per_core, (c + 1) * tiles_per_core))
        full, short = bs[:-1], bs[-1]
        full.sort(key=lambda b: -loads[b])
        for t, b in enumerate(full + [short]):
            base = c * per_core + t * 128
            ids = np.asarray(members[b], np.int64)
            perm_o2n[ids] = base + np.arange(len(ids))
    return perm_o2n


def _wrap16_rep(a):
    """[L] int -> [128, L/16] int16 (16-wrap, replicated 8x down partitions)."""
    w = a.reshape(-1, 16).T.astype(np.int16)
    return np.ascontiguousarray(np.tile(w, (8, 1)))


def _host_prep(inputs, n_cores=N_CORES):
    import ml_dtypes

    bf16 = ml_dtypes.bfloat16
    x = np.asarray(inputs["x"], np.float32)
    ei = np.asarray(inputs["edge_index"])
    n, f_in = x.shape
    loop = np.arange(n, dtype=ei.dtype)
    src = np.concatenate([ei[0], loop]).astype(np.int64)
    dst = np.concatenate([ei[1], loop]).astype(np.int64)

    per_core = n // n_cores
    T = math.ceil(per_core / 128)
    rows_last = per_core - (T - 1) * 128
    S = SPLIT_TILE
    rowsA = S * 128
    rowsB = per_core - rowsA

    perm = _balance_permutation(dst, n, n_cores, T, rows_last)

    local = perm % per_core
    core = perm // per_core
    in_a = local < rowsA
    grow = np.where(in_a, core * rowsA + local, core * rowsB + (local - rowsA))

    src_n = perm[src]
    dst_n = perm[dst]
    src_g = (~in_a[src]).astype(np.int64)      # 0 = group A, 1 = group B
    src_row = grow[src]                        # row within its group tensor

    core_of = dst_n // per_core
    loc_of = dst_n % per_core

    # per (core, tile, group): edge src rows + dst locals, sorted by dst
    per_ctg = {}
    counts = np.zeros((n_cores, T, 2), np.int64)
    for c in range(n_cores):
        sel = core_of == c
        s, loc, g = src_row[sel], loc_of[sel], src_g[sel]
        o = np.argsort(loc, kind="stable")
        s, loc, g = s[o], loc[o], g[o]
        tile_of = loc // 128
        for t in range(T):
            mt = tile_of == t
            for gg in range(2):
                m = mt & (g == gg)
                per_ctg[c, t, gg] = (s[m], loc[m] - t * 128)
                counts[c, t, gg] = m.sum()

    # per-tile ragged chunk counts, shared across cores (SPMD)
    nch = np.zeros((T, 2), np.int64)
    for t in range(T):
        for gg in range(2):
            nch[t, gg] = max(1, math.ceil(counts[:, t, gg].max() / 128))
    nchunks = [(int(nch[t, 0]), int(nch[t, 1])) for t in range(T)]
    ntot = [a + b for a, b in nchunks]

    co_idx = np.concatenate([[0], np.cumsum([v * 8 for v in ntot])])
    co_oh = np.concatenate([[0], np.cumsum([v * 128 for v in ntot])])

    src16 = np.zeros((n_cores, 128, int(co_idx[-1])), np.int16)
    ohs = np.zeros((n_cores, 128, int(co_oh[-1])), bf16)
    ohTs = np.zeros((n_cores, 128, int(co_oh[-1])), bf16)
    pp = np.arange(128)
    for c in range(n_cores):
        for t in range(T):
            nA, nB = nchunks[t]
            cap = (nA + nB) * 128
            ps = np.zeros(cap, np.int64)
            pl = np.full(cap, -1, np.int64)
            for gg, off, ncg in ((0, 0, nA), (1, nA * 128, nB)):
                s_, l_ = per_ctg[c, t, gg]
                e = len(s_)
                ps[off : off + e] = s_
                pl[off : off + e] = l_
            src16[c, :, co_idx[t] : co_idx[t + 1]] = _wrap16_rep(ps)
            dl = pl.reshape(nA + nB, 128)
            for cch in range(nA + nB):
                m = dl[cch] >= 0
                j = dl[cch, m]
                ohs[c, pp[m], co_oh[t] + cch * 128 + j] = 1
                ohTs[c, j, co_oh[t] + cch * 128 + pp[m]] = 1

    # permuted node features, transposed, padded rows, bf16, per core
    rows_pad = T * 128
    x_perm = np.zeros((n, f_in), np.float32)
    x_perm[perm] = x
    xT = []
    for c in range(n_cores):
        blk = np.zeros((rows_pad, f_in), np.float32)
        blk[:per_core] = x_perm[c * per_core : (c + 1) * per_core]
        xT.append(np.ascontiguousarray(blk.T).astype(bf16))

    g = lambda k: np.asarray(inputs[k], np.float32)
    h1, c1 = g("a1s").shape
    h3, c3 = g("a3s").shape
    d1 = h1 * c1

    def fold(Wk, ak_s, ak_d, h, cc):
        W = g(Wk)
        a_s, a_d = g(ak_s), g(ak_d)
        waS = np.stack([W[:, i * cc : (i + 1) * cc] @ a_s[i] for i in range(h)], 1)
        waD = np.stack([W[:, i * cc : (i + 1) * cc] @ a_d[i] for i in range(h)], 1)
        return np.concatenate([waS, waD], 1).astype(bf16)  # [din, 2h]

    waug1 = np.concatenate([g("W1"), g("Wl1")], 1).astype(bf16)   # [50, 2048]
    waug2 = np.concatenate([g("W2"), g("Wl2")], 1).astype(bf16)   # [1024, 2048]
    waug3 = np.concatenate([g("W3"), g("Wl3")], 1).astype(bf16)   # [1024, 847]
    wsd1 = fold("W1", "a1s", "a1d", h1, c1)
    wsd2 = fold("W2", "a2s", "a2d", h1, c1)
    wsd3 = fold("W3", "a3s", "a3d", h3, c3)

    rep = lambda v: np.ascontiguousarray(
        np.broadcast_to(v[None, :], (128, v.shape[0]))
    ).astype(np.float32)
    base = dict(
        waug1=waug1, waug2=waug2, waug3=waug3,
        wsd1=wsd1, wsd2=wsd2, wsd3=wsd3,
        bsum1=rep(g("b1") + g("bl1")),
        bsum2=rep(g("b2") + g("bl2")),
        bsum3=rep(g("b3") + g("bl3")),
        idmat=np.eye(128, dtype=bf16),
    )
    in_maps = []
    for c in range(n_cores):
        m = dict(base)
        m["xT1"] = xT[c]
        m["src16"] = src16[c]
        m["oh"] = np.ascontiguousarray(ohs[c])
        m["ohT"] = np.ascontiguousarray(ohTs[c])
        in_maps.append(m)

    cfg = dict(
        n=n, f_in=f_in, n_cores=n_cores, per_core=per_core,
        tiles_per_core=T, rows_last=rows_last, rows_pad=rows_pad,
        s_tile=S, rows_a=rowsA, rows_b=rowsB,
        nchunks=nchunks, co_idx=[int(v) for v in co_idx],
        co_oh=[int(v) for v in co_oh],
        h1=h1, c1=c1, d1=d1, h3=h3, c3=c3,
    )
    return in_maps, cfg, perm


# --------------------------------------------------------------------------
# bass program
# --------------------------------------------------------------------------

def _layer_dims(cfg):
    """Static per-layer dims. Payload row (bytes):
    [h0 | 1 | h1 | 1 | ... ] (H*(C+1)=HST fp8 bytes) then es (H f32),
    padded to a 256B multiple (dma_gather elem restriction)."""
    out = []
    for li in (1, 2, 3):
        if li < 3:
            h, c = cfg["h1"], cfg["c1"]
            din = cfg["f_in"] if li == 1 else cfg["d1"]
            nlin = cfg["d1"]
        else:
            h, c = cfg["h3"], cfg["c3"]
            din = cfg["d1"]
            nlin = cfg["c3"]
        st = c + 1
        hst = h * st
        assert hst % 4 == 0
        es = hst // 4                      # f32 index of es within the row
        pw = math.ceil((hst + 4 * h) / 256) * 256   # row bytes, 256B multiple
        kch = math.ceil(din / 128)
        hc = h * c
        out.append(dict(li=li, din=din, kch=kch, hc=hc, nlin=nlin,
                        h=h, c=c, st=st, hst=hst, es=es, pw=pw,
                        naug=hc + nlin))
    return out


def _groups(n, grp):
    """[(offset, size), ...] covering n chunks in groups of <= grp."""
    out = []
    o = 0
    while o < n:
        out.append((o, min(grp, n - o)))
        o += grp
    return out


def _build(cfg):
    import concourse.bass as bass
    import concourse.bacc as bacc
    import concourse.mybir as mybir
    import concourse.tile as tile
    from contextlib import ExitStack

    f32 = mybir.dt.float32
    bf = mybir.dt.bfloat16
    i16 = mybir.dt.int16
    u8 = mybir.dt.uint8
    f8 = mybir.dt.float8e4
    EXP = mybir.ActivationFunctionType.Exp
    CPY = mybir.ActivationFunctionType.Copy
    ALU = mybir.AluOpType

    n_cores = cfg["n_cores"]
    n = cfg["n"]
    T = cfg["tiles_per_core"]
    rows_last = cfg["rows_last"]
    per_core = cfg["per_core"]
    S = cfg["s_tile"]
    rowsA, rowsB = cfg["rows_a"], cfg["rows_b"]
    NCH = cfg["nchunks"]          # [(nA, nB)] per tile
    CO_IDX = cfg["co_idx"]
    CO_OH = cfg["co_oh"]
    NMAX = max(a + b for a, b in NCH)
    GRP = GROUP
    D1 = cfg["d1"]
    layers = _layer_dims(cfg)

    nc = bacc.Bacc(None, target_bir_lowering=False, num_swdge_queues=2)

    # ---- parameters -----------------------------------------------------
    xT1 = nc.declare_dram_parameter("xT1", [cfg["f_in"], T * 128], bf, isOutput=False)
    waug_p, wsd_p, bsum_p = {}, {}, {}
    for L in layers:
        li = L["li"]
        waug_p[li] = nc.declare_dram_parameter(
            f"waug{li}", [L["din"], L["naug"]], bf, isOutput=False)
        wsd_p[li] = nc.declare_dram_parameter(
            f"wsd{li}", [L["din"], 2 * L["h"]], bf, isOutput=False)
        bsum_p[li] = nc.declare_dram_parameter(
            f"bsum{li}", [128, L["nlin"]], f32, isOutput=False)
    src16_p = nc.declare_dram_parameter("src16", [128, CO_IDX[-1]], i16, isOutput=False)
    oh_p = nc.declare_dram_parameter("oh", [128, CO_OH[-1]], bf, isOutput=False)
    ohT_p = nc.declare_dram_parameter("ohT", [128, CO_OH[-1]], bf, isOutput=False)
    id_p = nc.declare_dram_parameter("idmat", [128, 128], bf, isOutput=False)
    out_p = nc.declare_dram_parameter("out", [per_core, cfg["c3"]], f32, isOutput=True)

    with tile.TileContext(nc, num_cores=n_cores) as tc, ExitStack() as ctx:
        # ---- dram scratch ----------------------------------------------
        dram = ctx.enter_context(tc.tile_pool(name="dram", bufs=1, space="DRAM"))
        pshard = {L["li"]: dram.tile([per_core, L["pw"]], u8, tag=f"pshard{L['li']}",
                                     name=f"pshard{L['li']}") for L in layers}
        pfullA = {L["li"]: dram.tile([n_cores * rowsA, L["pw"]], u8,
                                     tag=f"pfa{L['li']}", name=f"pfa{L['li']}",
                                     addr_space="Shared") for L in layers}
        pfullB = {L["li"]: dram.tile([n_cores * rowsB, L["pw"]], u8,
                                     tag=f"pfb{L['li']}", name=f"pfb{L['li']}",
                                     addr_space="Shared") for L in layers}
        linb = {L["li"]: dram.tile([T * 128, L["nlin"]], f32, tag=f"lin{L['li']}",
                                   name=f"lin{L['li']}") for L in layers}

        # ---- pools ------------------------------------------------------
        consts = ctx.enter_context(tc.tile_pool(name="consts", bufs=1))
        wtp = ctx.enter_context(tc.tile_pool(name="wtp", bufs=1))
        xTp = ctx.enter_context(tc.tile_pool(name="xTp", bufs=2))
        ptp = ctx.enter_context(tc.tile_pool(name="ptp", bufs=2))
        ltp = ctx.enter_context(tc.tile_pool(name="ltp", bufs=2))
        gp = ctx.enter_context(tc.tile_pool(name="gp", bufs=3))
        gwp = ctx.enter_context(tc.tile_pool(name="gwp", bufs=2))
        idxp = ctx.enter_context(tc.tile_pool(name="idxp", bufs=2))
        ohp = ctx.enter_context(tc.tile_pool(name="ohp", bufs=2))
        lgp = ctx.enter_context(tc.tile_pool(name="lgp", bufs=4))
        epip = ctx.enter_context(tc.tile_pool(name="epip", bufs=1))
        recp = ctx.enter_context(tc.tile_pool(name="recp", bufs=4))
        psum_d = ctx.enter_context(tc.tile_pool(name="psum_d", bufs=1, space="PSUM"))
        psum_a = ctx.enter_context(tc.tile_pool(name="psum_a", bufs=1, space="PSUM"))
        psum_e = ctx.enter_context(tc.tile_pool(name="psum_e", bufs=2, space="PSUM"))
        psum_t = ctx.enter_context(tc.tile_pool(name="psum_t", bufs=1, space="PSUM"))

        # ---- constants ---------------------------------------------------
        idm = consts.tile([128, 128], bf, tag="idm")
        nc.sync.dma_start(out=idm[:, :], in_=id_p[:, :])
        xT1_sb = consts.tile([cfg["f_in"], T * 128], bf, tag="xT1")
        nc.sync.dma_start(out=xT1_sb[:, :], in_=xT1[:, :])
        wt = {}
        wsd = {}
        bsum = {}
        edts = {}
        for L in layers:
            li, KCH, DIN = L["li"], L["kch"], L["din"]
            for k in range(KCH):
                kk = min(128, DIN - k * 128)
                w = wtp.tile([128, L["naug"]], bf, tag=f"w{li}_{k}", name=f"w{li}_{k}")
                nc.sync.dma_start(out=w[:kk, :], in_=waug_p[li][k * 128 : k * 128 + kk, :])
                wt[li, k] = w
                s = wtp.tile([128, 2 * L["h"]], bf, tag=f"s{li}_{k}", name=f"s{li}_{k}")
                nc.sync.dma_start(out=s[:kk, :], in_=wsd_p[li][k * 128 : k * 128 + kk, :])
                wsd[li, k] = s
            b = consts.tile([128, L["nlin"]], f32, tag=f"b{li}", name=f"b{li}")
            nc.sync.dma_start(out=b[:, :], in_=bsum_p[li][:, :])
            bsum[li] = b
            e = consts.tile([128, T * L["h"]], bf, tag=f"e{li}", name=f"e{li}")
            edts[li] = e

        qn = [0]  # round-robin dma queue counter

        def rows_of(t):
            return 128 if t < T - 1 else rows_last

        def chunks(w):
            return [(c0, min(c0 + 512, w)) for c0 in range(0, w, 512)]

        # ---------------- dense phase for one tile -----------------------
        def dense_tile(L, t, get_lhsT):
            li, DIN, KCH = L["li"], L["din"], L["kch"]
            H, C, ST, HST, HC = L["h"], L["c"], L["st"], L["hst"], L["hc"]
            NLIN, ES, PW = L["nlin"], L["es"], L["pw"]
            r = rows_of(t)

            lhsTs = {}

            def lhsT_of(k, kk):
                if k not in lhsTs:
                    lhsTs[k] = get_lhsT(k, kk)
                return lhsTs[k]

            wA = HC + NLIN if li == 3 else HC
            pse = psum_e.tile([128, 2 * H], f32, tag="pe", name="pse")
            pdA = psum_d.tile([128, 1024], f32, tag="pd", name="pdA")
            for k in range(KCH):
                kk = min(128, DIN - k * 128)
                lhsT = lhsT_of(k, kk)
                st, sp = (k == 0), (k == KCH - 1)
                for c0, c1 in chunks(wA):
                    nc.tensor.matmul(pdA[:, c0:c1], lhsT[:kk, :],
                                     wt[li, k][:kk, c0:c1], start=st, stop=sp)
                nc.tensor.matmul(pse[:, :], lhsT[:kk, :], wsd[li, k][:kk, :],
                                 start=st, stop=sp)

            # payload assembly (h in fp8, es in f32)
            pt = ptp.tile([128, PW], u8, tag="pt")
            ptb = pt.bitcast(f8)
            for h in range(H):
                nc.scalar.activation(ptb[:, h * ST : h * ST + C],
                                     pdA[:, h * C : (h + 1) * C], CPY)
            ones_v = ptb[:, :HST].rearrange("p (h s) -> p h s", h=H)[:, :, C : C + 1]
            nc.vector.memset(ones_v, 1.0)
            ptf = pt.bitcast(f32)
            nc.scalar.activation(ptf[:, ES : ES + H], pse[:, :H], CPY)
            nc.vector.memset(pt[:, HST + 4 * H : PW], 0)
            nc.scalar.activation(edts[li][:, t * H : (t + 1) * H], pse[:, H : 2 * H], CPY)
            nc.sync.dma_start(out=pshard[li][t * 128 : t * 128 + r, :], in_=pt[:r, :])
            # lin + bias staging (second pass reuses the pd psum slot for li<3)
            if li < 3:
                pdB = psum_d.tile([128, 1024], f32, tag="pd", name="pdB")
                for k in range(KCH):
                    kk = min(128, DIN - k * 128)
                    lhsT = lhsT_of(k, kk)
                    st, sp = (k == 0), (k == KCH - 1)
                    for c0, c1 in chunks(NLIN):
                        nc.tensor.matmul(pdB[:, c0:c1], lhsT[:kk, :],
                                         wt[li, k][:kk, HC + c0 : HC + c1],
                                         start=st, stop=sp)
            else:
                pdB = pdA
            loff = HC if li == 3 else 0
            lt = ltp.tile([128, NLIN], f32, tag="lt")
            nc.vector.tensor_tensor(out=lt[:, :], in0=pdB[:, loff : loff + NLIN],
                                    in1=bsum[li][:, :], op=ALU.add)
            nc.sync.dma_start(out=linb[li][t * 128 : t * 128 + r, :], in_=lt[:r, :])

        # ---------------- aggregation for one tile ------------------------
        def agg_tile(L, t):
            li = L["li"]
            H, C, ST, HST, HC = L["h"], L["c"], L["st"], L["hst"], L["hc"]
            NLIN, ES, PW = L["nlin"], L["es"], L["pw"]
            r = rows_of(t)
            nA, nB = NCH[t]
            NT = nA + nB

            s16 = idxp.tile([128, NMAX * 8], i16, tag="s16")
            nc.sync.dma_start(out=s16[:, : NT * 8],
                              in_=src16_p[:, CO_IDX[t] : CO_IDX[t + 1]])
            oh_sb = ohp.tile([128, NMAX * 128], bf, tag="oh")
            nc.sync.dma_start(out=oh_sb[:, : NT * 128],
                              in_=oh_p[:, CO_OH[t] : CO_OH[t + 1]])
            ohT_sb = ohp.tile([128, NMAX * 128], bf, tag="ohT")
            nc.sync.dma_start(out=ohT_sb[:, : NT * 128],
                              in_=ohT_p[:, CO_OH[t] : CO_OH[t + 1]])

            # ed per edge via ohT^T @ edt : [128 edges, H] per chunk
            pe = psum_e.tile([128, NMAX * H], f32, tag="pe", name="pe")
            for cch in range(NT):
                nc.tensor.matmul(pe[:, cch * H : (cch + 1) * H],
                                 ohT_sb[:, cch * 128 : (cch + 1) * 128],
                                 edts[li][:, t * H : (t + 1) * H],
                                 start=True, stop=True)

            pa = psum_a.tile([128, HST], f32, tag="pa", name="pa")
            glist = [(pfullA[li], o, sz) for o, sz in _groups(nA, GRP)]
            glist += [(pfullB[li], nA + o, sz) for o, sz in _groups(nB, GRP)]
            for src_t, goff, gsz in glist:
                G = gp.tile([128, GRP, PW], u8, tag="G")
                nc.gpsimd.dma_gather(
                    out_ap=G[:, :gsz, :],
                    in_ap=src_t[:, :],
                    idxs_ap=s16[:, goff * 8 : (goff + gsz) * 8],
                    num_idxs=gsz * 128,
                    num_idxs_reg=gsz * 128,
                    elem_size=PW,
                    queue_num=qn[0] % 2,
                )
                qn[0] += 1
                Gf = G.bitcast(f32)
                tl = lgp.tile([128, GRP, H], f32, tag="tl")
                wf = lgp.tile([128, GRP, H], f32, tag="wf")
                nc.vector.tensor_tensor(
                    out=tl[:, :gsz, :], in0=Gf[:, :gsz, ES : ES + H],
                    in1=pe[:, goff * H : (goff + gsz) * H].rearrange(
                        "p (g h) -> p g h", h=H),
                    op=ALU.add,
                )
                nc.vector.scalar_tensor_tensor(
                    out=wf[:, :gsz, :], in0=tl[:, :gsz, :], scalar=0.2,
                    in1=tl[:, :gsz, :], op0=ALU.mult, op1=ALU.max,
                )
                web = lgp.tile([128, GRP, H], bf, tag="web")
                nc.scalar.activation(web[:, :gsz, :], wf[:, :gsz, :], EXP)
                G8 = G.bitcast(f8)[:, :gsz, :HST].rearrange(
                    "p g (h s) -> p g h s", h=H)
                webB = web[:, :gsz].unsqueeze(3).broadcast_to([128, gsz, H, ST])
                Gw = gwp.tile([128, GRP, HST], bf, tag="Gw")
                nc.vector.tensor_tensor(
                    out=Gw[:, :gsz].rearrange("p g (h s) -> p g h s", h=H),
                    in0=G8, in1=webB, op=ALU.mult)
                for cch in range(gsz):
                    j = goff + cch
                    for c0, c1 in chunks(HST):
                        nc.tensor.matmul(
                            pa[:, c0:c1],
                            oh_sb[:, j * 128 : (j + 1) * 128],
                            Gw[:, cch, c0:c1],
                            start=(j == 0),
                            stop=(j == NT - 1),
                        )

            # ---- epilogue (pa drained to SBUF first to free PSUM) ----
            pac = epip.tile([128, HST], f32, tag="pac", bufs=2)
            nc.scalar.activation(pac[:, :], pa[:, :], CPY)
            xt = epip.tile([128, HC], f32, tag="xt", bufs=2)
            for h in range(H):
                dn = recp.tile([128, 1], f32, tag="dn")
                nc.vector.tensor_scalar(out=dn[:, :],
                                        in0=pac[:, h * ST + C : h * ST + C + 1],
                                        scalar1=1e-30, scalar2=None, op0=ALU.max)
                rec = recp.tile([128, 1], f32, tag="rec")
                nc.vector.reciprocal(rec[:, :], dn[:, :])
                nc.vector.tensor_scalar(
                    out=xt[:, h * C : (h + 1) * C], in0=pac[:, h * ST : h * ST + C],
                    scalar1=rec[:, 0:1], scalar2=None, op0=ALU.mult,
                )
            lt2 = ltp.tile([128, NLIN], f32, tag="lt2")
            nc.sync.dma_start(out=lt2[:r, :], in_=linb[li][t * 128 : t * 128 + r, :])
            if li < 3:
                u = epip.tile([128, HC], f32, tag="u")
                e = epip.tile([128, HC], f32, tag="e")
                xo = epip.tile([128, HC], bf, tag="xo")
                if r < 128:
                    nc.vector.memset(xo[:, :], 0)
                nc.vector.tensor_tensor(out=xt[:r, :], in0=xt[:r, :], in1=lt2[:r, :],
                                        op=ALU.add)
                nc.vector.tensor_scalar(out=u[:r, :], in0=xt[:r, :], scalar1=0.0,
                                        scalar2=None, op0=ALU.min)
                nc.scalar.activation(e[:r, :], u[:r, :], EXP)
                nc.vector.tensor_scalar(out=xt[:r, :], in0=xt[:r, :], scalar1=0.0,
                                        scalar2=-1.0, op0=ALU.max, op1=ALU.add)
                nc.vector.tensor_tensor(out=xo[:r, :], in0=xt[:r, :], in1=e[:r, :],
                                        op=ALU.add)
                # PE transpose -> next layer lhsT in SBUF
                ptr = psum_t.tile([128, HC], bf, tag="ptr", name="ptr")
                for k in range(HC // 128):
                    nc.tensor.transpose(ptr[:, k * 128 : (k + 1) * 128],
                                        xo[:, k * 128 : (k + 1) * 128], idm[:, :])
                xT_sb = xTp.tile([128, HC], bf, tag="xT")
                nc.scalar.activation(xT_sb[:, :], ptr[:, :], CPY)
                return xT_sb
            else:
                xt3 = xt.rearrange("p (h c) -> p h c", h=H)
                hh = H // 2
                m1 = epip.tile([128, hh, C], f32, tag="m1")
                nc.vector.tensor_tensor(out=m1[:, :, :], in0=xt3[:, 0:hh, :],
                                        in1=xt3[:, hh : 2 * hh, :], op=ALU.add)
                m2 = epip.tile([128, C], f32, tag="m2")
                nc.vector.tensor_tensor(out=m2[:, :], in0=m1[:, 0, :], in1=m1[:, 1, :],
                                        op=ALU.add)
                for i in range(2, hh):
                    nc.vector.tensor_tensor(out=m2[:, :], in0=m2[:, :], in1=m1[:, i, :],
                                            op=ALU.add)
                ot = epip.tile([128, C], f32, tag="ot")
                nc.vector.scalar_tensor_tensor(
                    out=ot[:r, :], in0=m2[:r, :], scalar=1.0 / H, in1=lt2[:r, :],
                    op0=ALU.mult, op1=ALU.add,
                )
                nc.sync.dma_start(out=out_p[t * 128 : t * 128 + r, :], in_=ot[:r, :])
                return None

        # ------------------------------------------------------------------
        def maybe_ag(li, t):
            if t == S - 1:
                nc.gpsimd.collective_compute(
                    "AllGather", ALU.bypass,
                    replica_groups=[list(range(n_cores))],
                    ins=[pshard[li][0:rowsA, :].opt()],
                    outs=[pfullA[li].opt()],
                )
            elif t == T - 1:
                nc.gpsimd.collective_compute(
                    "AllGather", ALU.bypass,
                    replica_groups=[list(range(n_cores))],
                    ins=[pshard[li][rowsA:per_core, :].opt()],
                    outs=[pfullB[li].opt()],
                )

        # ---- layer 1 dense (lhsT slices preloaded xT1) --------------------
        def l1_lhsT_for(t):
            def get(k, kk):
                return xT1_sb[:, t * 128 : (t + 1) * 128]
            return get

        for t in range(T):
            dense_tile(layers[0], t, l1_lhsT_for(t))
            maybe_ag(1, t)

        # ---- agg L1 + dense L2, agg L2 + dense L3, agg L3 -----------------
        for t in range(T):
            xT_sb = agg_tile(layers[0], t)
            dense_tile(layers[1], t, lambda k, kk, x=xT_sb: x[:, k * 128 : (k + 1) * 128])
            maybe_ag(2, t)
        for t in range(T):
            xT_sb = agg_tile(layers[1], t)
            dense_tile(layers[2], t, lambda k, kk, x=xT_sb: x[:, k * 128 : (k + 1) * 128])
            maybe_ag(3, t)
        for t in range(T):
            agg_tile(layers[2], t)

    nc.finalize()
    return nc


# --------------------------------------------------------------------------
# runner
# --------------------------------------------------------------------------

def _run(inputs, sim=False, trace=False, n_cores=N_CORES, tmpdir=None):
    in_maps, cfg, perm = _host_prep(inputs, n_cores)
    nc = _build(cfg)
    if sim:
        import concourse.bass_interp as bass_interp

        msim = bass_interp.MultiCoreSim(nc, n_cores)
        for c in range(n_cores):
            for k, v in in_maps[c].items():
                msim.cores[c].tensor(k)[:] = v
        msim.simulate(check_with_hw=True)
        outs = [np.array(msim.cores[c].mem_tensor("out")) for c in range(n_cores)]
        exec_ns = None
    else:
        from concourse.bass_utils import run_bass_kernel_spmd

        res = run_bass_kernel_spmd(
            nc, in_maps, list(range(n_cores)), trace=trace, tmpdir=tmpdir
        )
        outs = [res.results[c]["out"] for c in range(n_cores)]
        exec_ns = res.exec_time_ns
    out_new = np.concatenate(outs, 0)       # rows in (core, local) order
    out = np.empty_like(out_new)
    out[...] = out_new[perm]
    return out.astype(np.float32), exec_ns


def kernel(**inputs) -> np.ndarray:
    out, _ = _run(inputs)
    return out


# revision 40
# speedup vs baseline: 1.1318x; 1.0600x over previous
"""3-layer GAT (PPI-style) forward on 8 Trainium2 NeuronCores — v3.

Strategy (SPMD, one NEFF on 8 cores):
  - Host: add self-loops, degree-balanced node permutation into 8 cores x
    2500 nodes (tiles of 128 dst nodes); per-core tiles sorted by edge load
    so that the per-tile-index max across cores is minimal; edges sorted by
    dst, chunked to 128-edge chunks with PER-TILE ragged chunk counts;
    int16 gather-index arrays and STATIC one-hot scatter matrices
    precomputed (flattened ragged layout).
  - Payload AllGather is SPLIT in two (tiles [0,S) and [S,T)) on separate
    Shared tensors so AG-A overlaps the tail of the producing loop; edges
    are src-partitioned into the two groups with separate gathers.
  - Dense phase per tile: [h | lin] = x @ [W | Wl] (bf16, PE); es/ed
    attention dots folded into the matmul as extra columns; payload row
    [h0|1|h1|1|...|es] (fp8 + f32 es tail) staged to DRAM shard.
  - Aggregation per tile: ed via tiny matmuls ohT^T @ edt; payload
    dma_gather per group; w = exp(leaky(es+ed)) scales the payload with a
    broadcast multiply; scatter-sum via matmul with the static one-hot
    lhsT (512-col chunks, PSUM-accumulated); PSUM drained early via scalar
    copy; normalize, add skip+bias, ELU; PE-transpose keeps the next
    layer's lhsT in SBUF; next layer's dense phase inlined per tile.
"""

import math
import numpy as np

N_CORES = 8
GROUP = 3          # max gather chunks per dma_gather call (small groups
                   # keep the PE fed in ~3us pieces -> full clock p-state)
SPLIT_TILE = 16    # tiles [0, S) -> AG group A, [S, T) -> group B


# --------------------------------------------------------------------------
# host-side prep (data layout / graph partitioning / static one-hots)
# --------------------------------------------------------------------------

def _balance_permutation(dst, n, n_cores, tiles_per_core, rows_last):
    """Greedy balance: nodes -> 128-row dst tiles with ~equal edge counts.
    Within each core, full tiles are then ordered by load (desc) so the
    max-over-cores load per tile index stays near the mean."""
    import heapq

    deg = np.bincount(dst, minlength=n).astype(np.int64)
    order = np.argsort(-deg, kind="stable")
    n_tiles = n_cores * tiles_per_core
    caps = np.full(n_tiles, 128, np.int64)
    caps[tiles_per_core - 1 :: tiles_per_core] = rows_last
    heap = [(0, int(b)) for b in range(n_tiles)]
    heapq.heapify(heap)
    members = [[] for _ in range(n_tiles)]
    loads = np.zeros(n_tiles, np.int64)
    for node in order:
        while True:
            load, b = heapq.heappop(heap)
            if len(members[b]) < caps[b]:
                break
        members[b].append(node)
        loads[b] += deg[node]
        if len(members[b]) < caps[b]:
            heapq.heappush(heap, (int(loads[b]), b))

    perm_o2n = np.empty(n, np.int64)
    per_core = tiles_per_core * 128 - (128 - rows_last)
    for c in range(n_cores):
        bs = list(range(c * tiles_per_core, (c + 1) * tiles_per_core))
        full, short = bs[:-1], bs[-1]
        full.sort(key=lambda b: -loads[b])
        for t, b in enumerate(full + [short]):
            base = c * per_core + t * 128
            ids = np.asarray(members[b], np.int64)
            perm_o2n[ids] = base + np.arange(len(ids))
    return perm_o2n


def _wrap16_rep(a):
    """[L] int -> [128, L/16] int16 (16-wrap, replicated 8x down partitions)."""
    w = a.reshape(-1, 16).T.astype(np.int16)
    return np.ascontiguousarray(np.tile(w, (8, 1)))


def _host_prep(inputs, n_cores=N_CORES):
    import ml_dtypes

    bf16 = ml_dtypes.bfloat16
    x = np.asarray(inputs["x"], np.float32)
    ei = np.asarray(inputs["edge_index"])
    n, f_in = x.shape
    loop = np.arange(n, dtype=ei.dtype)
    src = np.concatenate([ei[0], loop]).astype(np.int64)
    dst = np.concatenate([ei[1], loop]).astype(np.int64)

    per_core = n // n_cores
    T = math.ceil(per_core / 128)
    rows_last = per_core - (T - 1) * 128
    S = SPLIT_TILE
    rowsA = S * 128
    rowsB = per_core - rowsA

    perm = _balance_permutation(dst, n, n_cores, T, rows_last)

    local = perm % per_core
    core = perm // per_core
    in_a = local < rowsA
    grow = np.where(in_a, core * rowsA + local, core * rowsB + (local - rowsA))

    src_n = perm[src]
    dst_n = perm[dst]
    src_g = (~in_a[src]).astype(np.int64)      # 0 = group A, 1 = group B
    src_row = grow[src]                        # row within its group tensor

    core_of = dst_n // per_core
    loc_of = dst_n % per_core

    # per (core, tile, group): edge src rows + dst locals, sorted by dst
    per_ctg = {}
    counts = np.zeros((n_cores, T, 2), np.int64)
    for c in range(n_cores):
        sel = core_of == c
        s, loc, g = src_row[sel], loc_of[sel], src_g[sel]
        o = np.argsort(loc, kind="stable")
        s, loc, g = s[o], loc[o], g[o]
        tile_of = loc // 128
        for t in range(T):
            mt = tile_of == t
            for gg in range(2):
                m = mt & (g == gg)
                per_ctg[c, t, gg] = (s[m], loc[m] - t * 128)
                counts[c, t, gg] = m.sum()

    # per-tile ragged chunk counts, shared across cores (SPMD)
    nch = np.zeros((T, 2), np.int64)
    for t in range(T):
        for gg in range(2):
            nch[t, gg] = max(1, math.ceil(counts[:, t, gg].max() / 128))
    nchunks = [(int(nch[t, 0]), int(nch[t, 1])) for t in range(T)]
    ntot = [a + b for a, b in nchunks]

    co_idx = np.concatenate([[0], np.cumsum([v * 8 for v in ntot])])
    co_oh = np.concatenate([[0], np.cumsum([v * 128 for v in ntot])])

    src16 = np.zeros((n_cores, 128, int(co_idx[-1])), np.int16)
    ohs = np.zeros((n_cores, 128, int(co_oh[-1])), bf16)
    ohTs = np.zeros((n_cores, 128, int(co_oh[-1])), bf16)
    pp = np.arange(128)
    for c in range(n_cores):
        for t in range(T):
            nA, nB = nchunks[t]
            cap = (nA + nB) * 128
            ps = np.zeros(cap, np.int64)
            pl = np.full(cap, -1, np.int64)
            for gg, off, ncg in ((0, 0, nA), (1, nA * 128, nB)):
                s_, l_ = per_ctg[c, t, gg]
                e = len(s_)
                ps[off : off + e] = s_
                pl[off : off + e] = l_
            src16[c, :, co_idx[t] : co_idx[t + 1]] = _wrap16_rep(ps)
            dl = pl.reshape(nA + nB, 128)
            for cch in range(nA + nB):
                m = dl[cch] >= 0
                j = dl[cch, m]
                ohs[c, pp[m], co_oh[t] + cch * 128 + j] = 1
                ohTs[c, j, co_oh[t] + cch * 128 + pp[m]] = 1

    # permuted node features, transposed, padded rows, bf16, per core
    rows_pad = T * 128
    x_perm = np.zeros((n, f_in), np.float32)
    x_perm[perm] = x
    xT = []
    for c in range(n_cores):
        blk = np.zeros((rows_pad, f_in), np.float32)
        blk[:per_core] = x_perm[c * per_core : (c + 1) * per_core]
        xT.append(np.ascontiguousarray(blk.T).astype(bf16))

    g = lambda k: np.asarray(inputs[k], np.float32)
    h1, c1 = g("a1s").shape
    h3, c3 = g("a3s").shape
    d1 = h1 * c1

    def fold(Wk, ak_s, ak_d, h, cc):
        W = g(Wk)
        a_s, a_d = g(ak_s), g(ak_d)
        waS = np.stack([W[:, i * cc : (i + 1) * cc] @ a_s[i] for i in range(h)], 1)
        waD = np.stack([W[:, i * cc : (i + 1) * cc] @ a_d[i] for i in range(h)], 1)
        return np.concatenate([waS, waD], 1).astype(bf16)  # [din, 2h]

    waug1 = np.concatenate([g("W1"), g("Wl1")], 1).astype(bf16)   # [50, 2048]
    waug2 = np.concatenate([g("W2"), g("Wl2")], 1).astype(bf16)   # [1024, 2048]
    waug3 = np.concatenate([g("W3"), g("Wl3")], 1).astype(bf16)   # [1024, 847]
    wsd1 = fold("W1", "a1s", "a1d", h1, c1)
    wsd2 = fold("W2", "a2s", "a2d", h1, c1)
    wsd3 = fold("W3", "a3s", "a3d", h3, c3)

    rep = lambda v: np.ascontiguousarray(
        np.broadcast_to(v[None, :], (128, v.shape[0]))
    ).astype(np.float32)
    base = dict(
        waug1=waug1, waug2=waug2, waug3=waug3,
        wsd1=wsd1, wsd2=wsd2, wsd3=wsd3,
        bsum1=rep(g("b1") + g("bl1")),
        bsum2=rep(g("b2") + g("bl2")),
        bsum3=rep(g("b3") + g("bl3")),
        idmat=np.eye(128, dtype=bf16),
    )
    in_maps = []
    for c in range(n_cores):
        m = dict(base)
        m["xT1"] = xT[c]
        m["src16"] = src16[c]
        m["oh"] = np.ascontiguousarray(ohs[c])
        m["ohT"] = np.ascontiguousarray(ohTs[c])
        in_maps.append(m)

    cfg = dict(
        n=n, f_in=f_in, n_cores=n_cores, per_core=per_core,
        tiles_per_core=T, rows_last=rows_last, rows_pad=rows_pad,
        s_tile=S, rows_a=rowsA, rows_b=rowsB,
        nchunks=nchunks, co_idx=[int(v) for v in co_idx],
        co_oh=[int(v) for v in co_oh],
        h1=h1, c1=c1, d1=d1, h3=h3, c3=c3,
    )
    return in_maps, cfg, perm


# --------------------------------------------------------------------------
# bass program
# --------------------------------------------------------------------------

def _layer_dims(cfg):
    """Static per-layer dims. Payload row (bytes):
    [h0 | 1 | h1 | 1 | ... ] (H*(C+1)=HST fp8 bytes) then es (H f32),
    padded to a 256B multiple (dma_gather elem restriction)."""
    out = []
    for li in (1, 2, 3):
        if li < 3:
            h, c = cfg["h1"], cfg["c1"]
            din = cfg["f_in"] if li == 1 else cfg["d1"]
            nlin = cfg["d1"]
        else:
            h, c = cfg["h3"], cfg["c3"]
            din = cfg["d1"]
            nlin = cfg["c3"]
        st = c + 1
        hst = h * st
        assert hst % 4 == 0
        es = hst // 4                      # f32 index of es within the row
        pw = math.ceil((hst + 4 * h) / 256) * 256   # row bytes, 256B multiple
        kch = math.ceil(din / 128)
        hc = h * c
        out.append(dict(li=li, din=din, kch=kch, hc=hc, nlin=nlin,
                        h=h, c=c, st=st, hst=hst, es=es, pw=pw,
                        naug=hc + nlin))
    return out


def _groups(n, grp):
    """[(offset, size), ...] covering n chunks in groups of <= grp."""
    out = []
    o = 0
    while o < n:
        out.append((o, min(grp, n - o)))
        o += grp
    return out


def _build(cfg):
    import concourse.bass as bass
    import concourse.bacc as bacc
    import concourse.mybir as mybir
    import concourse.tile as tile
    from contextlib import ExitStack

    f32 = mybir.dt.float32
    bf = mybir.dt.bfloat16
    i16 = mybir.dt.int16
    u8 = mybir.dt.uint8
    f8 = mybir.dt.float8e4
    EXP = mybir.ActivationFunctionType.Exp
    CPY = mybir.ActivationFunctionType.Copy
    ALU = mybir.AluOpType

    n_cores = cfg["n_cores"]
    n = cfg["n"]
    T = cfg["tiles_per_core"]
    rows_last = cfg["rows_last"]
    per_core = cfg["per_core"]
    S = cfg["s_tile"]
    rowsA, rowsB = cfg["rows_a"], cfg["rows_b"]
    NCH = cfg["nchunks"]          # [(nA, nB)] per tile
    CO_IDX = cfg["co_idx"]
    CO_OH = cfg["co_oh"]
    NMAX = max(a + b for a, b in NCH)
    GRP = GROUP
    D1 = cfg["d1"]
    layers = _layer_dims(cfg)

    nc = bacc.Bacc(None, target_bir_lowering=False, num_swdge_queues=2)

    # ---- parameters -----------------------------------------------------
    xT1 = nc.declare_dram_parameter("xT1", [cfg["f_in"], T * 128], bf, isOutput=False)
    waug_p, wsd_p, bsum_p = {}, {}, {}
    for L in layers:
        li = L["li"]
        waug_p[li] = nc.declare_dram_parameter(
            f"waug{li}", [L["din"], L["naug"]], bf, isOutput=False)
        wsd_p[li] = nc.declare_dram_parameter(
            f"wsd{li}", [L["din"], 2 * L["h"]], bf, isOutput=False)
        bsum_p[li] = nc.declare_dram_parameter(
            f"bsum{li}", [128, L["nlin"]], f32, isOutput=False)
    src16_p = nc.declare_dram_parameter("src16", [128, CO_IDX[-1]], i16, isOutput=False)
    oh_p = nc.declare_dram_parameter("oh", [128, CO_OH[-1]], bf, isOutput=False)
    ohT_p = nc.declare_dram_parameter("ohT", [128, CO_OH[-1]], bf, isOutput=False)
    id_p = nc.declare_dram_parameter("idmat", [128, 128], bf, isOutput=False)
    out_p = nc.declare_dram_parameter("out", [per_core, cfg["c3"]], f32, isOutput=True)

    with tile.TileContext(nc, num_cores=n_cores) as tc, ExitStack() as ctx:
        # ---- dram scratch ----------------------------------------------
        dram = ctx.enter_context(tc.tile_pool(name="dram", bufs=1, space="DRAM"))
        pshard = {L["li"]: dram.tile([per_core, L["pw"]], u8, tag=f"pshard{L['li']}",
                                     name=f"pshard{L['li']}") for L in layers}
        pfullA = {L["li"]: dram.tile([n_cores * rowsA, L["pw"]], u8,
                                     tag=f"pfa{L['li']}", name=f"pfa{L['li']}",
                                     addr_space="Shared") for L in layers}
        pfullB = {L["li"]: dram.tile([n_cores * rowsB, L["pw"]], u8,
                                     tag=f"pfb{L['li']}", name=f"pfb{L['li']}",
                                     addr_space="Shared") for L in layers}
        linb = {L["li"]: dram.tile([T * 128, L["nlin"]], f32, tag=f"lin{L['li']}",
                                   name=f"lin{L['li']}") for L in layers}

        # ---- pools ------------------------------------------------------
        consts = ctx.enter_context(tc.tile_pool(name="consts", bufs=1))
        wtp = ctx.enter_context(tc.tile_pool(name="wtp", bufs=1))
        xTp = ctx.enter_context(tc.tile_pool(name="xTp", bufs=2))
        ptp = ctx.enter_context(tc.tile_pool(name="ptp", bufs=2))
        ltp = ctx.enter_context(tc.tile_pool(name="ltp", bufs=2))
        gp = ctx.enter_context(tc.tile_pool(name="gp", bufs=5))
        gwp = ctx.enter_context(tc.tile_pool(name="gwp", bufs=4))
        idxp = ctx.enter_context(tc.tile_pool(name="idxp", bufs=2))
        ohp = ctx.enter_context(tc.tile_pool(name="ohp", bufs=2))
        lgp = ctx.enter_context(tc.tile_pool(name="lgp", bufs=4))
        epip = ctx.enter_context(tc.tile_pool(name="epip", bufs=1))
        recp = ctx.enter_context(tc.tile_pool(name="recp", bufs=4))
        psum_d = ctx.enter_context(tc.tile_pool(name="psum_d", bufs=1, space="PSUM"))
        psum_a = ctx.enter_context(tc.tile_pool(name="psum_a", bufs=1, space="PSUM"))
        psum_e = ctx.enter_context(tc.tile_pool(name="psum_e", bufs=2, space="PSUM"))
        psum_t = ctx.enter_context(tc.tile_pool(name="psum_t", bufs=1, space="PSUM"))

        # ---- constants ---------------------------------------------------
        idm = consts.tile([128, 128], bf, tag="idm")
        nc.sync.dma_start(out=idm[:, :], in_=id_p[:, :])
        xT1_sb = consts.tile([cfg["f_in"], T * 128], bf, tag="xT1")
        nc.sync.dma_start(out=xT1_sb[:, :], in_=xT1[:, :])
        wt = {}
        wsd = {}
        bsum = {}
        edts = {}
        for L in layers:
            li, KCH, DIN = L["li"], L["kch"], L["din"]
            for k in range(KCH):
                kk = min(128, DIN - k * 128)
                w = wtp.tile([128, L["naug"]], bf, tag=f"w{li}_{k}", name=f"w{li}_{k}")
                nc.sync.dma_start(out=w[:kk, :], in_=waug_p[li][k * 128 : k * 128 + kk, :])
                wt[li, k] = w
                s = wtp.tile([128, 2 * L["h"]], bf, tag=f"s{li}_{k}", name=f"s{li}_{k}")
                nc.sync.dma_start(out=s[:kk, :], in_=wsd_p[li][k * 128 : k * 128 + kk, :])
                wsd[li, k] = s
            b = consts.tile([128, L["nlin"]], f32, tag=f"b{li}", name=f"b{li}")
            nc.sync.dma_start(out=b[:, :], in_=bsum_p[li][:, :])
            bsum[li] = b
            e = consts.tile([128, T * L["h"]], bf, tag=f"e{li}", name=f"e{li}")
            edts[li] = e

        qn = [0]  # round-robin dma queue counter

        def rows_of(t):
            return 128 if t < T - 1 else rows_last

        def chunks(w):
            return [(c0, min(c0 + 512, w)) for c0 in range(0, w, 512)]

        # ---------------- dense phase for one tile -----------------------
        def dense_tile(L, t, get_lhsT):
            li, DIN, KCH = L["li"], L["din"], L["kch"]
            H, C, ST, HST, HC = L["h"], L["c"], L["st"], L["hst"], L["hc"]
            NLIN, ES, PW = L["nlin"], L["es"], L["pw"]
            r = rows_of(t)

            lhsTs = {}

            def lhsT_of(k, kk):
                if k not in lhsTs:
                    lhsTs[k] = get_lhsT(k, kk)
                return lhsTs[k]

            wA = HC + NLIN if li == 3 else HC
            pse = psum_e.tile([128, 2 * H], f32, tag="pe", name="pse")
            pdA = psum_d.tile([128, 1024], f32, tag="pd", name="pdA")
            for k in range(KCH):
                kk = min(128, DIN - k * 128)
                lhsT = lhsT_of(k, kk)
                st, sp = (k == 0), (k == KCH - 1)
                for c0, c1 in chunks(wA):
                    nc.tensor.matmul(pdA[:, c0:c1], lhsT[:kk, :],
                                     wt[li, k][:kk, c0:c1], start=st, stop=sp)
                nc.tensor.matmul(pse[:, :], lhsT[:kk, :], wsd[li, k][:kk, :],
                                 start=st, stop=sp)

            # payload assembly (h in fp8, es in f32)
            pt = ptp.tile([128, PW], u8, tag="pt")
            ptb = pt.bitcast(f8)
            for h in range(H):
                nc.scalar.activation(ptb[:, h * ST : h * ST + C],
                                     pdA[:, h * C : (h + 1) * C], CPY)
            ones_v = ptb[:, :HST].rearrange("p (h s) -> p h s", h=H)[:, :, C : C + 1]
            nc.vector.memset(ones_v, 1.0)
            ptf = pt.bitcast(f32)
            nc.scalar.activation(ptf[:, ES : ES + H], pse[:, :H], CPY)
            nc.vector.memset(pt[:, HST + 4 * H : PW], 0)
            nc.scalar.activation(edts[li][:, t * H : (t + 1) * H], pse[:, H : 2 * H], CPY)
            nc.sync.dma_start(out=pshard[li][t * 128 : t * 128 + r, :], in_=pt[:r, :])
            # lin + bias staging (second pass reuses the pd psum slot for li<3)
            if li < 3:
                pdB = psum_d.tile([128, 1024], f32, tag="pd", name="pdB")
                for k in range(KCH):
                    kk = min(128, DIN - k * 128)
                    lhsT = lhsT_of(k, kk)
                    st, sp = (k == 0), (k == KCH - 1)
                    for c0, c1 in chunks(NLIN):
                        nc.tensor.matmul(pdB[:, c0:c1], lhsT[:kk, :],
                                         wt[li, k][:kk, HC + c0 : HC + c1],
                                         start=st, stop=sp)
            else:
                pdB = pdA
            loff = HC if li == 3 else 0
            lt = ltp.tile([128, NLIN], f32, tag="lt")
            nc.vector.tensor_tensor(out=lt[:, :], in0=pdB[:, loff : loff + NLIN],
                                    in1=bsum[li][:, :], op=ALU.add)
            nc.sync.dma_start(out=linb[li][t * 128 : t * 128 + r, :], in_=lt[:r, :])

        # ---------------- aggregation for one tile ------------------------
        def agg_tile(L, t):
            li = L["li"]
            H, C, ST, HST, HC = L["h"], L["c"], L["st"], L["hst"], L["hc"]
            NLIN, ES, PW = L["nlin"], L["es"], L["pw"]
            r = rows_of(t)
            nA, nB = NCH[t]
            NT = nA + nB

            s16 = idxp.tile([128, NMAX * 8], i16, tag="s16")
            nc.sync.dma_start(out=s16[:, : NT * 8],
                              in_=src16_p[:, CO_IDX[t] : CO_IDX[t + 1]])
            oh_sb = ohp.tile([128, NMAX * 128], bf, tag="oh")
            nc.sync.dma_start(out=oh_sb[:, : NT * 128],
                              in_=oh_p[:, CO_OH[t] : CO_OH[t + 1]])
            ohT_sb = ohp.tile([128, NMAX * 128], bf, tag="ohT")
            nc.sync.dma_start(out=ohT_sb[:, : NT * 128],
                              in_=ohT_p[:, CO_OH[t] : CO_OH[t + 1]])

            # ed per edge via ohT^T @ edt : [128 edges, H] per chunk
            pe = psum_e.tile([128, NMAX * H], f32, tag="pe", name="pe")
            for cch in range(NT):
                nc.tensor.matmul(pe[:, cch * H : (cch + 1) * H],
                                 ohT_sb[:, cch * 128 : (cch + 1) * 128],
                                 edts[li][:, t * H : (t + 1) * H],
                                 start=True, stop=True)

            pa = psum_a.tile([128, HST], f32, tag="pa", name="pa")
            glist = [(pfullA[li], o, sz) for o, sz in _groups(nA, GRP)]
            glist += [(pfullB[li], nA + o, sz) for o, sz in _groups(nB, GRP)]
            for src_t, goff, gsz in glist:
                G = gp.tile([128, GRP, PW], u8, tag="G")
                nc.gpsimd.dma_gather(
                    out_ap=G[:, :gsz, :],
                    in_ap=src_t[:, :],
                    idxs_ap=s16[:, goff * 8 : (goff + gsz) * 8],
                    num_idxs=gsz * 128,
                    num_idxs_reg=gsz * 128,
                    elem_size=PW,
                    queue_num=qn[0] % 2,
                )
                qn[0] += 1
                Gf = G.bitcast(f32)
                tl = lgp.tile([128, GRP, H], f32, tag="tl")
                wf = lgp.tile([128, GRP, H], f32, tag="wf")
                nc.vector.tensor_tensor(
                    out=tl[:, :gsz, :], in0=Gf[:, :gsz, ES : ES + H],
                    in1=pe[:, goff * H : (goff + gsz) * H].rearrange(
                        "p (g h) -> p g h", h=H),
                    op=ALU.add,
                )
                nc.vector.scalar_tensor_tensor(
                    out=wf[:, :gsz, :], in0=tl[:, :gsz, :], scalar=0.2,
                    in1=tl[:, :gsz, :], op0=ALU.mult, op1=ALU.max,
                )
                web = lgp.tile([128, GRP, H], bf, tag="web")
                nc.scalar.activation(web[:, :gsz, :], wf[:, :gsz, :], EXP)
                G8 = G.bitcast(f8)[:, :gsz, :HST].rearrange(
                    "p g (h s) -> p g h s", h=H)
                webB = web[:, :gsz].unsqueeze(3).broadcast_to([128, gsz, H, ST])
                Gw = gwp.tile([128, GRP, HST], bf, tag="Gw")
                nc.vector.tensor_tensor(
                    out=Gw[:, :gsz].rearrange("p g (h s) -> p g h s", h=H),
                    in0=G8, in1=webB, op=ALU.mult)
                for cch in range(gsz):
                    j = goff + cch
                    for c0, c1 in chunks(HST):
                        nc.tensor.matmul(
                            pa[:, c0:c1],
                            oh_sb[:, j * 128 : (j + 1) * 128],
                            Gw[:, cch, c0:c1],
                            start=(j == 0),
                            stop=(j == NT - 1),
                        )

            # ---- epilogue (pa drained to SBUF first to free PSUM) ----
            pac = epip.tile([128, HST], f32, tag="pac", bufs=2)
            nc.scalar.activation(pac[:, :], pa[:, :], CPY)
            xt = epip.tile([128, HC], f32, tag="xt", bufs=2)
            for h in range(H):
                dn = recp.tile([128, 1], f32, tag="dn")
                nc.vector.tensor_scalar(out=dn[:, :],
                                        in0=pac[:, h * ST + C : h * ST + C + 1],
                                        scalar1=1e-30, scalar2=None, op0=ALU.max)
                rec = recp.tile([128, 1], f32, tag="rec")
                nc.vector.reciprocal(rec[:, :], dn[:, :])
                nc.vector.tensor_scalar(
                    out=xt[:, h * C : (h + 1) * C], in0=pac[:, h * ST : h * ST + C],
                    scalar1=rec[:, 0:1], scalar2=None, op0=ALU.mult,
                )
            lt2 = ltp.tile([128, NLIN], f32, tag="lt2")
            nc.sync.dma_start(out=lt2[:r, :], in_=linb[li][t * 128 : t * 128 + r, :])
            if li < 3:
                u = epip.tile([128, HC], f32, tag="u")
                e = epip.tile([128, HC], f32, tag="e")
                xo = epip.tile([128, HC], bf, tag="xo")
                if r < 128:
                    nc.vector.memset(xo[:, :], 0)
                nc.vector.tensor_tensor(out=xt[:r, :], in0=xt[:r, :], in1=lt2[:r, :],
                                        op=ALU.add)
                nc.vector.tensor_scalar(out=u[:r, :], in0=xt[:r, :], scalar1=0.0,
                                        scalar2=None, op0=ALU.min)
                nc.scalar.activation(e[:r, :], u[:r, :], EXP)
                nc.vector.tensor_scalar(out=xt[:r, :], in0=xt[:r, :], scalar1=0.0,
                                        scalar2=-1.0, op0=ALU.max, op1=ALU.add)
                nc.vector.tensor_tensor(out=xo[:r, :], in0=xt[:r, :], in1=e[:r, :],
                                        op=ALU.add)
                # PE transpose -> next layer lhsT in SBUF
                ptr = psum_t.tile([128, HC], bf, tag="ptr", name="ptr")
                for k in range(HC // 128):
                    nc.tensor.transpose(ptr[:, k * 128 : (k + 1) * 128],
                                        xo[:, k * 128 : (k + 1) * 128], idm[:, :])
                xT_sb = xTp.tile([128, HC], bf, tag="xT")
                nc.scalar.activation(xT_sb[:, :], ptr[:, :], CPY)
                return xT_sb
            else:
                xt3 = xt.rearrange("p (h c) -> p h c", h=H)
                hh = H // 2
                m1 = epip.tile([128, hh, C], f32, tag="m1")
                nc.vector.tensor_tensor(out=m1[:, :, :], in0=xt3[:, 0:hh, :],
                                        in1=xt3[:, hh : 2 * hh, :], op=ALU.add)
                m2 = epip.tile([128, C], f32, tag="m2")
                nc.vector.tensor_tensor(out=m2[:, :], in0=m1[:, 0, :], in1=m1[:, 1, :],
                                        op=ALU.add)
                for i in range(2, hh):
                    nc.vector.tensor_tensor(out=m2[:, :], in0=m2[:, :], in1=m1[:, i, :],
                                            op=ALU.add)
                ot = epip.tile([128, C], f32, tag="ot")
                nc.vector.scalar_tensor_tensor(
                    out=ot[:r, :], in0=m2[:r, :], scalar=1.0 / H, in1=lt2[:r, :],
                    op0=ALU.mult, op1=ALU.add,
                )
                nc.sync.dma_start(out=out_p[t * 128 : t * 128 + r, :], in_=ot[:r, :])
                return None

        # ------------------------------------------------------------------
        def maybe_ag(li, t):
            if t == S - 1:
                nc.gpsimd.collective_compute(
                    "AllGather", ALU.bypass,
                    replica_groups=[list(range(n_cores))],
                    ins=[pshard[li][0:rowsA, :].opt()],
                    outs=[pfullA[li].opt()],
                )
            elif t == T - 1:
                nc.gpsimd.collective_compute(
                    "AllGather", ALU.bypass,
                    replica_groups=[list(range(n_cores))],
                    ins=[pshard[li][rowsA:per_core, :].opt()],
                    outs=[pfullB[li].opt()],
                )

        # ---- layer 1 dense (lhsT slices preloaded xT1) --------------------
        def l1_lhsT_for(t):
            def get(k, kk):
                return xT1_sb[:, t * 128 : (t + 1) * 128]
            return get

        for t in range(T):
            dense_tile(layers[0], t, l1_lhsT_for(t))
            maybe_ag(1, t)

        # ---- agg L1 + dense L2, agg L2 + dense L3, agg L3 -----------------
        for t in range(T):
            xT_sb = agg_tile(layers[0], t)
            dense_tile(layers[1], t, lambda k, kk, x=xT_sb: x[:, k * 128 : (k + 1) * 128])
            maybe_ag(2, t)
        for t in range(T):
            xT_sb = agg_tile(layers[1], t)
            dense_tile(layers[2], t, lambda k, kk, x=xT_sb: x[:, k * 128 : (k + 1) * 128])
            maybe_ag(3, t)
        for t in range(T):
            agg_tile(layers[2], t)

    nc.finalize()
    return nc


# --------------------------------------------------------------------------
# runner
# --------------------------------------------------------------------------

def _run(inputs, sim=False, trace=False, n_cores=N_CORES, tmpdir=None):
    in_maps, cfg, perm = _host_prep(inputs, n_cores)
    nc = _build(cfg)
    if sim:
        import concourse.bass_interp as bass_interp

        msim = bass_interp.MultiCoreSim(nc, n_cores)
        for c in range(n_cores):
            for k, v in in_maps[c].items():
                msim.cores[c].tensor(k)[:] = v
        msim.simulate(check_with_hw=True)
        outs = [np.array(msim.cores[c].mem_tensor("out")) for c in range(n_cores)]
        exec_ns = None
    else:
        from concourse.bass_utils import run_bass_kernel_spmd

        res = run_bass_kernel_spmd(
            nc, in_maps, list(range(n_cores)), trace=trace, tmpdir=tmpdir
        )
        outs = [res.results[c]["out"] for c in range(n_cores)]
        exec_ns = res.exec_time_ns
    out_new = np.concatenate(outs, 0)       # rows in (core, local) order
    out = np.empty_like(out_new)
    out[...] = out_new[perm]
    return out.astype(np.float32), exec_ns


def kernel(**inputs) -> np.ndarray:
    out, _ = _run(inputs)
    return out
